# revision 9
# baseline (speedup 1.0000x reference)
"""MultiHeadAttention Trainium2 Bass kernel (B=8, S=1024, D=1024, H=16).

Sharding: data-parallel over batch — core b computes batch element b.

v2 schedule rework. Per-core algorithm (matmuls bf16, PSUM fp32):
  * Host pre-tiles every input into block-ordered contiguous pieces so the
    DMA stream can be priority-ordered: wq_r0, xq_sc0, wk_r0, xk_sc0 (the
    4.5MB prefix that unblocks the first scores+exp at ~7us), then xk_sc1,
    xq_sc1, pair-1 weights, wv/xv (V projection inputs), then the remaining
    wq/wk r-blocks.  Pieces are issued 2-deep with junk-read gates so they
    complete in priority order at full bandwidth.
  * Projections on PE: Q^T/K^T r-pair chains (8 k-steps, [128,512] PSUM),
    V chains xv-stationary x wv-moving -> VA [kpos, d] with a ones column
    per head (softmax denominator via the attnV matmul).
  * Scores: per (r, qc, c): scores^T[kpos, q] = K_h x Q_h^T, two heads
    packed via tile_position row groups (K=64 each), PSUM [128, 1024].
  * Softmax: ONE ScalarE exp per step, -1e9 key mask fused via the
    per-partition bias port, bf16 P^T out.  The exp stream is the pacer
    (~1.11us/step); every other engine is scheduled to hide under it.
  * attnV: pt-block stationary x [V_h | ones]-moving, accumulated over c
    in PSUM [128, 4, 65] per head.  attnV chunks are scheduled by a static
    greedy table that respects exp completion, V-projection progress and
    PSUM residency; early groups are staggered, later groups catch up at
    ~1.3 chunks/step.
  * Out-phase: bulk-copy po PSUM->SBUF on DVE (frees the psum bank fast),
    reciprocal on DVE, normalization muls on the idle gpsimd engine,
    DMA out from gpsimd.
"""
import numpy as np
import ml_dtypes

import concourse.bass as bass
import concourse.mybir as mybir
import concourse.tile as tile
from concourse.bass_utils import run_bass_kernel_spmd

F32 = mybir.dt.float32
BF16 = mybir.dt.bfloat16
AF = mybir.ActivationFunctionType

B, S, D, H = 8, 1024, 1024, 16
DH = D // H          # 64
KT = 8               # contraction chunks of 128
NEG = -1.0e9
N_CORES = 8

_cache = {}


def _split_excess_waits(nc, limit: int = 1):
    """Walrus TPB instruction structs encode exactly ONE wait; hoist excess
    waits emitted by Tile into standalone InstEventSemaphore instructions."""
    ctr = 0
    for f in nc.m.functions:
        for bb in f.blocks:
            new = []
            changed = False
            for inst in bb.instructions:
                si = inst.sync_info
                waits = list(si.on_wait) if si is not None and si.on_wait else []
                if len(waits) > limit:
                    excess, keep = waits[:-limit], waits[-limit:]
                    for w in excess:
                        ctr += 1
                        new.append(mybir.InstEventSemaphore(
                            name=f"wsplit-{ctr}",
                            engine=inst.engine,
                            ins=[], outs=[],
                            sync_info=mybir.SyncInfo(on_wait=[w], on_update=[]),
                        ))
                    inst.sync_info = mybir.SyncInfo(
                        on_wait=keep,
                        on_update=list(si.on_update) if si.on_update else [],
                    )
                    changed = True
                new.append(inst)
            if changed:
                bb.instructions = new
    return ctr


def _attnv_schedule():
    """step -> list of (g, c) attnV chunk slots.  Static greedy: respects
    exp completion (chunk (g,c) after step 8g+c), V-projection progress
    (VA st=c complete ~step 6 + 16(c+1)/6), and keeps groups compact so at
    most 2 po accumulators are open (pod pool bufs=3)."""
    va_ready = [6 + (16 * (c + 1) + 5) // 6 for c in range(8)]   # st=c done
    sched = {}
    pend = [(g, c) for g in range(16) for c in range(8)]
    i = 0
    for s in range(0, 300):
        if i >= len(pend):
            break
        cap = 0
        if s >= 10:
            cap = 1
        if s >= 36 and s % 3 == 0:
            cap = 2
        if s >= 128:                       # drain steps: no scores/exp
            cap = 3
        got = []
        for _ in range(cap):
            if i >= len(pend):
                break
            g, c = pend[i]
            if 8 * g + c >= s:             # exp not done yet
                break
            if va_ready[c] > s:            # V projection not landed
                break
            got.append((g, c))
            i += 1
        if got:
            sched[s] = got
    assert i == len(pend), f"attnV schedule incomplete: {i}/128"
    return sched


def _build_program():
    nc = bass.Bass()
    # pre-tiled inputs (bf16): see _prep_inputs for layouts
    xq_p = [nc.declare_dram_parameter(f"xq{i}", [128, 8 * 512], BF16,
                                      isOutput=False) for i in range(2)]
    xk_p = [nc.declare_dram_parameter(f"xk{i}", [128, 8 * 512], BF16,
                                      isOutput=False) for i in range(2)]
    wv_p = [nc.declare_dram_parameter(f"wv{i}", [128, 8 * 512], BF16,
                                      isOutput=False) for i in range(2)]
    xv_p = [nc.declare_dram_parameter(f"xv{i}", [128, 8 * 128], BF16,
                                      isOutput=False) for i in range(8)]
    wq_p = [nc.declare_dram_parameter(f"wq{i}", [128, 8 * 128], BF16,
                                      isOutput=False) for i in range(8)]
    wk_p = [nc.declare_dram_parameter(f"wk{i}", [128, 8 * 128], BF16,
                                      isOutput=False) for i in range(8)]
    msk = nc.declare_dram_parameter("msk", [128, KT], F32, isOutput=False)
    idn = nc.declare_dram_parameter("idn", [128, 128], BF16, isOutput=False)
    out = nc.declare_dram_parameter("out", [S, D], F32, isOutput=True)

    att_sched = _attnv_schedule()
    # pt buffer depth = max (exps emitted) - (chunks consumed) + margin
    need = 0
    consumed = 0
    for s in range(200):
        consumed += len(att_sched.get(s, []))
        emitted = min(s + 1, 128)
        need = max(need, emitted - consumed)
    ptp_bufs = need + 2

    with tile.TileContext(nc) as tc:
        with (
            tc.tile_pool(name="persist", bufs=1) as pers,
            tc.tile_pool(name="pt", bufs=ptp_bufs) as ptp,
            tc.tile_pool(name="outp", bufs=3) as outp,
            tc.tile_pool(name="rr", bufs=4) as rrp,
            tc.tile_pool(name="pp", bufs=2, space="PSUM") as pp,
            tc.tile_pool(name="psc", bufs=2, space="PSUM") as psc,
            tc.tile_pool(name="pod", bufs=2, space="PSUM") as pod,
        ):
            # ---------- constants (tiny, front of scalar queue) ----------
            mask_sb = pers.tile([128, KT], F32)
            nc.scalar.dma_start(out=mask_sb, in_=msk[:, :])
            id_sb = pers.tile([128, 128], BF16)
            nc.scalar.dma_start(out=id_sb, in_=idn[:, :])

            # ---------- persistent input pieces ----------
            XQ = [pers.tile([128, 8, 512], BF16, name=f"XQ{i}")
                  for i in range(2)]
            XK = [pers.tile([128, 8, 512], BF16, name=f"XK{i}")
                  for i in range(2)]
            WV = [pers.tile([128, 8, 512], BF16, name=f"WV{i}")
                  for i in range(2)]
            XV = [pers.tile([128, 8, 128], BF16, name=f"XV{i}")
                  for i in range(8)]
            WQ = [pers.tile([128, 8, 128], BF16, name=f"WQ{i}")
                  for i in range(8)]
            WK = [pers.tile([128, 8, 128], BF16, name=f"WK{i}")
                  for i in range(8)]

            def ld(dst, param, w):
                nc_eng = nc.scalar if w == 0 else nc.gpsimd
                nc_eng.dma_start(
                    out=dst, in_=param[:, :].rearrange("p (k s) -> p k s",
                                                       s=dst.shape[2]))

            junk_s = pers.tile([128, 1], BF16)
            junk_g = pers.tile([1, 2], BF16)
            warm = pers.tile([128, 1], F32)
            nc.scalar.copy(warm, mask_sb[:, 0:1])            # warm ACT clock

            # scalar queue: critical prefix, 2-deep with gates so pieces
            # complete in priority order at full bandwidth.
            ld(WQ[0], wq_p[0], 0)
            ld(XQ[0], xq_p[0], 0)
            nc.scalar.copy(junk_s, WQ[0][:, 0, 0:1])
            ld(WK[0], wk_p[0], 0)
            nc.scalar.copy(junk_s, XQ[0][:, 0, 0:1])
            ld(XK[0], xk_p[0], 0)
            # (XK0 gate is the gpsimd handoff below; scalar then runs exps)

            # gpsimd queue: the rest, gated 2-deep; first gate waits for the
            # scalar prefix so these don't steal its bandwidth.
            nc.gpsimd.tensor_copy(junk_g, XK[0][0:1, 0, 0:2])
            ld(XK[1], xk_p[1], 1)
            ld(XQ[1], xq_p[1], 1)
            nc.gpsimd.tensor_copy(junk_g, XK[1][0:1, 0, 0:2])
            ld(WQ[1], wq_p[1], 1)
            ld(WK[1], wk_p[1], 1)
            ld(WV[0], wv_p[0], 1)
            nc.gpsimd.tensor_copy(junk_g, XQ[1][0:1, 0, 0:2])
            for i in range(4):
                ld(XV[i], xv_p[i], 1)
            nc.gpsimd.tensor_copy(junk_g, WV[0][0:1, 0, 0:2])
            ld(WV[1], wv_p[1], 1)
            for i in range(4, 8):
                ld(XV[i], xv_p[i], 1)
            nc.gpsimd.tensor_copy(junk_g, XV[3][0:1, 0, 0:2])
            for r in (2, 3):
                ld(WQ[r], wq_p[r], 1)
                ld(WK[r], wk_p[r], 1)
            nc.gpsimd.tensor_copy(junk_g, XV[7][0:1, 0, 0:2])
            for r in (4, 5):
                ld(WQ[r], wq_p[r], 1)
                ld(WK[r], wk_p[r], 1)
            nc.gpsimd.tensor_copy(junk_g, WK[3][0:1, 0, 0:2])
            for r in (6, 7):
                ld(WQ[r], wq_p[r], 1)
                ld(WK[r], wk_p[r], 1)

            # HAM warm-up: junk matmuls while the first pieces stream, so
            # the first projection matmuls run at 2.4 GHz
            for _ in range(4):
                jw = pp.tile([32, 128], F32, tag="pp")
                for j in range(9):
                    nc.tensor.matmul(
                        jw[:, :], lhsT=id_sb[0:32, 0:32],
                        rhs=id_sb[0:32, 0:128],
                        start=(j == 0), stop=(j == 8))

            # ---------- persistent activations ----------
            QT = pers.tile([128, KT, S], BF16)     # Q^T tiles: rows 128r+p
            KTt = pers.tile([128, KT, S], BF16)    # K^T
            VA = pers.tile([128, KT, H * 65], BF16)  # V with ones columns

            def wq_s(k, r):
                return WQ[r][:, k, :]

            def wk_s(k, r):
                return WK[r][:, k, :]

            def xq_s(k, sc):
                return XQ[sc][:, k, :]

            def xk_s(k, sc):
                return XK[sc][:, k, :]

            def xv_s(k, st):
                return XV[st][:, k, :]

            def wv_s(k, dc):
                return WV[dc][:, k, :]

            def va_slices(st, dc):
                dst = VA[:, st, :].rearrange("p (h w) -> p h w", w=65)
                return dst[:, dc * 8:(dc + 1) * 8, 0:64]

            class ProjStepper:
                """Emit projection chains one matmul at a time so they pace
                evenly between attention steps.

                Chain specs:
                  ('qk', which, sc, r)   -- 8 k-steps, full contraction
                  ('v8', st, dc)         -- 8 k-steps, V projection
                """

                def __init__(self, chains, pool, tag, max_active=1):
                    self.pending = list(chains)
                    self.active = []   # [psum_tile, chain_spec, next_j]
                    self.rr = 0
                    self.pool, self.tag = pool, tag
                    self.MAX_ACTIVE = max_active

                def _start(self):
                    if self.pending:
                        spec = self.pending.pop(0)
                        pq = self.pool.tile([128, 512], F32, tag=self.tag,
                                            name=f"pq_{'_'.join(map(str, spec))}")
                        self.active.append([pq, spec, 0])

                def step(self, n=2):
                    for _ in range(n):
                        while len(self.active) < self.MAX_ACTIVE and self.pending:
                            self._start()
                        if not self.active:
                            return
                        ent = self.active[self.rr % len(self.active)]
                        self.rr += 1
                        pq, spec, j = ent
                        if spec[0] == 'qk':
                            _, which, sc, r = spec
                            w_s, x_s = ((wq_s, xq_s) if which == 0
                                        else (wk_s, xk_s))
                            nc.tensor.matmul(
                                pq[:, :], lhsT=w_s(j, r), rhs=x_s(j, sc),
                                start=(j == 0), stop=(j == KT - 1))
                        else:
                            _, st, dc = spec
                            nc.tensor.matmul(
                                pq[:, :], lhsT=xv_s(j, st), rhs=wv_s(j, dc),
                                start=(j == 0), stop=(j == KT - 1))
                        ent[2] += 1
                        if ent[2] == KT:
                            if spec[0] == 'qk':
                                _, which, sc, r = spec
                                dstT = QT if which == 0 else KTt
                                nc.vector.tensor_copy(
                                    dstT[:, r, bass.ts(sc, 512)], pq)
                            else:
                                _, st, dc = spec
                                nc.vector.tensor_copy(
                                    va_slices(st, dc),
                                    pq[:, :].rearrange("p (h w) -> p h w",
                                                       w=64))
                                if dc == 1:
                                    ones = VA[:, st, :].rearrange(
                                        "p (h w) -> p h w", w=65)
                                    nc.vector.memset(ones[:, :, 64:65], 1.0)
                            self.active.remove(ent)

                def finish(self):
                    while self.active or self.pending:
                        self.step(1)

            # prelude: QK pair-0 sc0 chains (first scores need only these);
            # 2-active so consecutive matmuls alternate psum banks.
            pre = ProjStepper([('qk', 1, 0, 0), ('qk', 0, 0, 0)],
                              pp, "pp", max_active=2)
            pre.finish()

            # main-loop steppers
            sc1 = ProjStepper([('qk', 1, 1, 0), ('qk', 0, 1, 0)], pp, "pp")
            pair = {r: ProjStepper([('qk', w, sc, r)
                                    for w in (0, 1) for sc in (0, 1)],
                                   pp, "pp") for r in range(1, 8)}
            vstep = ProjStepper([('v8', st, dc)
                                 for st in range(8) for dc in range(2)],
                                pp, "pp")

            def pace(s):
                if s < 16:
                    pair[1].step(2)
                if 1 <= s <= 4:
                    sc1.step(4)
                if 6 <= s <= 27:
                    vstep.step(6)
                r_next = s // 16 + 1        # pair r+1 paced over pair r
                if 16 <= s and r_next <= 7:
                    pair[r_next].step(2)
                if s == 27:
                    vstep.finish()
                if s == 111:
                    for r in range(2, 8):
                        pair[r].finish()

            OPs = {}

            def scores_exp(r, qc, c):
                ps = psc.tile([128, 1024], F32, tag="psc")
                nc.tensor.matmul(
                    ps[:, 0:512],
                    lhsT=KTt[0:64, r, bass.ts(c, 128)],
                    rhs=QT[0:64, r, bass.ts(qc, 512)],
                    start=True, stop=True, tile_position=(0, 0))
                nc.tensor.matmul(
                    ps[:, 512:1024],
                    lhsT=KTt[64:128, r, bass.ts(c, 128)],
                    rhs=QT[64:128, r, bass.ts(qc, 512)],
                    start=True, stop=True, tile_position=(64, 0))
                pt = ptp.tile([128, 1024], BF16, tag="pt")
                nc.scalar.activation(pt, ps, AF.Exp,
                                     bias=mask_sb[:, c:c + 1], scale=1.0)
                return pt

            def attnv_chunk(r, po1, po2, pt, c):
                for ph, (po, hh) in enumerate(((po1, 2 * r), (po2, 2 * r + 1))):
                    for qt in range(4):
                        nc.tensor.matmul(
                            po[:, qt, :],
                            lhsT=pt[:, ph * 512 + qt * 128:
                                    ph * 512 + (qt + 1) * 128],
                            rhs=VA[:, c, hh * 65:(hh + 1) * 65],
                            start=(c == 0 and qt == 0),
                            stop=(c == KT - 1))

            def outphase(r, qc, po1, po2, tail):
                rh = r // 4
                OP = OPs.get((rh, qc))
                if OP is None:
                    OP = OPs[(rh, qc)] = outp.tile(
                        [128, 4, 512], BF16, tag="outp", name=f"OP{rh}_{qc}")
                # bulk-drain po psum fast (pod rotates every 1.5 groups),
                # then normalize on the idle gpsimd engine out of SBUF.
                sb1 = rrp.tile([128, 4, 65], F32, tag="sbp", bufs=4,
                               name=f"sb1_{r}_{qc}")
                sb2 = rrp.tile([128, 4, 65], F32, tag="sbp", bufs=4,
                               name=f"sb2_{r}_{qc}")
                nc.vector.tensor_copy(sb1, po1)
                nc.vector.tensor_copy(sb2, po2)
                rr1 = rrp.tile([128, 4, 1], F32, tag="rr", bufs=4,
                               name=f"rr1_{r}_{qc}")
                rr2 = rrp.tile([128, 4, 1], F32, tag="rr", bufs=4,
                               name=f"rr2_{r}_{qc}")
                nc.vector.reciprocal(rr1, sb1[:, :, 64:65])
                nc.vector.reciprocal(rr2, sb2[:, :, 64:65])
                for qt in range(4):
                    for ph, (sb, rr) in enumerate(((sb1, rr1), (sb2, rr2))):
                        nc.gpsimd.tensor_scalar_mul(
                            OP[:, qt, (r % 4) * 128 + ph * DH:
                               (r % 4) * 128 + (ph + 1) * DH],
                            sb[:, qt, 0:64], rr[:, qt, 0:1])
                    if tail:
                        nc.gpsimd.dma_start(
                            out=out[qc * 512 + qt * 128:
                                    qc * 512 + (qt + 1) * 128,
                                    bass.ts(rh, 512)],
                            in_=OP[:, qt, :])
                if r % 4 == 3 and not tail:
                    nc.gpsimd.dma_start(
                        out=out[bass.ts(qc, 512), bass.ts(rh, 512)].rearrange(
                            "(a p) w -> p a w", p=128),
                        in_=OP[:, :, :])
                    OPs[(rh, qc)] = None

            # ---- main loop ----
            pts_store = {}
            po_store = {}
            n_done = 0
            for s in range(300):
                if s < 128:
                    r, qc, c = s // 16, (s // 8) % 2, s % 8
                    pts_store[(s // 8, c)] = scores_exp(r, qc, c)
                    pace(s)
                for (g, cc) in att_sched.get(s, []):
                    gr, gqc = g // 2, g % 2
                    if cc == 0:
                        hp1 = pod.tile([128, 4, 65], F32, tag="pod",
                                       name=f"po1_g{g}")
                        hp2 = pod.tile([128, 4, 65], F32, tag="pod",
                                       name=f"po2_g{g}")
                        po_store[g] = (hp1, hp2)
                    hp1, hp2 = po_store[g]
                    attnv_chunk(gr, hp1, hp2, pts_store.pop((g, cc)), cc)
                    if cc == KT - 1:
                        outphase(gr, gqc, hp1, hp2, tail=(g == 15))
                        del po_store[g]
                        n_done += 1
                if s >= 128 and n_done == 16:
                    break

    _split_excess_waits(nc)
    return nc


def _retile(a, blk):
    """[D, D]-like [k*128+p, n] -> list of pieces [128, 8*blk] where piece i
    holds columns [i*blk, (i+1)*blk), layout p-major then (k, col)."""
    D0, D1 = a.shape
    out = []
    t = a.reshape(8, 128, D1)
    for i in range(D1 // blk):
        piece = t[:, :, i * blk:(i + 1) * blk].transpose(1, 0, 2)
        out.append(np.ascontiguousarray(piece.reshape(128, 8 * blk)))
    return out


def _prep_inputs(queries, keys, values, valid_lens, w_q, w_k, w_v):
    bf = ml_dtypes.bfloat16
    wq_b = (w_q.astype(np.float32) / np.sqrt(DH)).astype(bf)
    wk_b = w_k.astype(np.float32).astype(bf)
    wv_b = w_v.astype(np.float32).astype(bf)
    wq_t = _retile(wq_b, 128)
    wk_t = _retile(wk_b, 128)
    wv_t = _retile(wv_b, 512)
    idn = np.eye(128, dtype=bf)
    in_maps = []
    for b in range(B):
        mask = np.where(np.arange(S) < int(valid_lens[b]), 0.0, NEG)
        mask = np.ascontiguousarray(
            mask.reshape(KT, 128).T.astype(np.float32))          # [128, KT]
        xq_t = _retile(queries[b].astype(np.float32).T.astype(bf), 512)
        xk_t = _retile(keys[b].astype(np.float32).T.astype(bf), 512)
        xv_t = _retile(values[b].astype(np.float32).T.astype(bf), 128)
        m = dict(msk=mask, idn=idn)
        for i in range(2):
            m[f"xq{i}"] = xq_t[i]
            m[f"xk{i}"] = xk_t[i]
            m[f"wv{i}"] = wv_t[i]
        for i in range(8):
            m[f"xv{i}"] = xv_t[i]
            m[f"wq{i}"] = wq_t[i]
            m[f"wk{i}"] = wk_t[i]
        in_maps.append(m)
    return in_maps


def kernel(queries, keys, values, valid_lens, w_q, w_k, w_v, _want_results=False):
    queries = np.asarray(queries)
    keys = np.asarray(keys)
    values = np.asarray(values)
    valid_lens = np.asarray(valid_lens)
    w_q, w_k, w_v = np.asarray(w_q), np.asarray(w_k), np.asarray(w_v)
    if "nc" not in _cache:
        _cache["nc"] = _build_program()
    nc = _cache["nc"]
    in_maps = _prep_inputs(queries, keys, values, valid_lens, w_q, w_k, w_v)
    res = run_bass_kernel_spmd(nc, in_maps, list(range(N_CORES)))
    out = np.stack([res.results[b]["out"] for b in range(B)]).astype(np.float32)
    # valid_len == 0: reference softmaxes an all -1e9 row -> uniform attention.
    for b in range(B):
        if int(valid_lens[b]) == 0:
            vfull = values[b].astype(np.float32) @ w_v.astype(np.float32)
            out[b] = np.broadcast_to(vfull.mean(axis=0), (S, D))
    if _want_results:
        return out, res
    return out


# revision 12
# speedup vs baseline: 1.0420x; 1.0420x over previous
"""MultiHeadAttention Trainium2 Bass kernel (B=8, S=1024, D=1024, H=16).

Sharding: data-parallel over batch — core b computes batch element b.

v2 schedule rework. Per-core algorithm (matmuls bf16, PSUM fp32):
  * Host pre-tiles every input into block-ordered contiguous pieces so the
    DMA stream can be priority-ordered: wq_r0, xq_sc0, wk_r0, xk_sc0 (the
    4.5MB prefix that unblocks the first scores+exp at ~7us), then xk_sc1,
    xq_sc1, pair-1 weights, wv/xv (V projection inputs), then the remaining
    wq/wk r-blocks.  Pieces are issued 2-deep with junk-read gates so they
    complete in priority order at full bandwidth.
  * Projections on PE: Q^T/K^T r-pair chains (8 k-steps, [128,512] PSUM),
    V chains xv-stationary x wv-moving -> VA [kpos, d] with a ones column
    per head (softmax denominator via the attnV matmul).
  * Scores: per (r, qc, c): scores^T[kpos, q] = K_h x Q_h^T, two heads
    packed via tile_position row groups (K=64 each), PSUM [128, 1024].
  * Softmax: ONE ScalarE exp per step, -1e9 key mask fused via the
    per-partition bias port, bf16 P^T out.  The exp stream is the pacer
    (~1.11us/step); every other engine is scheduled to hide under it.
  * attnV: pt-block stationary x [V_h | ones]-moving, accumulated over c
    in PSUM [128, 4, 65] per head.  attnV chunks are scheduled by a static
    greedy table that respects exp completion, V-projection progress and
    PSUM residency; early groups are staggered, later groups catch up at
    ~1.3 chunks/step.
  * Out-phase: bulk-copy po PSUM->SBUF on DVE (frees the psum bank fast),
    reciprocal on DVE, normalization muls on the idle gpsimd engine,
    DMA out from gpsimd.
"""
import numpy as np
import ml_dtypes

import concourse.bass as bass
import concourse.mybir as mybir
import concourse.tile as tile
from concourse.bass_utils import run_bass_kernel_spmd

F32 = mybir.dt.float32
BF16 = mybir.dt.bfloat16
AF = mybir.ActivationFunctionType

B, S, D, H = 8, 1024, 1024, 16
DH = D // H          # 64
KT = 8               # contraction chunks of 128
NEG = -1.0e9
N_CORES = 8

_cache = {}


def _split_excess_waits(nc, limit: int = 1):
    """Walrus TPB instruction structs encode exactly ONE wait; hoist excess
    waits emitted by Tile into standalone InstEventSemaphore instructions."""
    ctr = 0
    for f in nc.m.functions:
        for bb in f.blocks:
            new = []
            changed = False
            for inst in bb.instructions:
                si = inst.sync_info
                waits = list(si.on_wait) if si is not None and si.on_wait else []
                if len(waits) > limit:
                    excess, keep = waits[:-limit], waits[-limit:]
                    for w in excess:
                        ctr += 1
                        new.append(mybir.InstEventSemaphore(
                            name=f"wsplit-{ctr}",
                            engine=inst.engine,
                            ins=[], outs=[],
                            sync_info=mybir.SyncInfo(on_wait=[w], on_update=[]),
                        ))
                    inst.sync_info = mybir.SyncInfo(
                        on_wait=keep,
                        on_update=list(si.on_update) if si.on_update else [],
                    )
                    changed = True
                new.append(inst)
            if changed:
                bb.instructions = new
    return ctr


def _attnv_schedule():
    """step -> list of (g, c) attnV chunk slots.  Static greedy: respects
    exp completion (chunk (g,c) after step 8g+c), V-projection progress
    (VA st=c complete ~step 6 + 16(c+1)/6), and keeps groups compact so at
    most 2 po accumulators are open (pod pool bufs=3)."""
    va_ready = [6 + (16 * (c + 1) + 5) // 6 for c in range(8)]   # st=c done
    sched = {}
    pend = [(g, c) for g in range(16) for c in range(8)]
    i = 0
    last_close = -10                       # step of the last (g, 7) chunk
    for s in range(0, 300):
        if i >= len(pend):
            break
        cap = 0
        if s >= 10:
            cap = 1
        if s >= 36 and s % 2 == 0:
            cap = 2
        if s >= 128:                       # drain steps: no scores/exp
            cap = 3
        got = []
        for _ in range(cap):
            if i >= len(pend):
                break
            g, c = pend[i]
            if 8 * g + c >= s:             # exp not done yet
                break
            if va_ready[c] > s:            # V projection not landed
                break
            # keep >=2 steps between closing group g and opening g+1: the
            # pod psum slots alias one group apart (bufs=2), so attnV(g+1)
            # c0 must wait for outphase(g)'s bulk psum->SBUF drain.
            if c == 0 and s < last_close + 2:
                break
            got.append((g, c))
            i += 1
            if c == KT - 1:
                last_close = s
                break                      # never open g+1 in the same step
        if got:
            sched[s] = got
    assert i == len(pend), f"attnV schedule incomplete: {i}/128"
    return sched


def _build_program():
    nc = bass.Bass()
    # pre-tiled inputs (bf16): see _prep_inputs for layouts
    xq_p = [nc.declare_dram_parameter(f"xq{i}", [128, 8 * 512], BF16,
                                      isOutput=False) for i in range(2)]
    xk_p = [nc.declare_dram_parameter(f"xk{i}", [128, 8 * 512], BF16,
                                      isOutput=False) for i in range(2)]
    wv_p = [nc.declare_dram_parameter(f"wv{i}", [128, 8 * 512], BF16,
                                      isOutput=False) for i in range(2)]
    xv_p = [nc.declare_dram_parameter(f"xv{i}", [128, 8 * 128], BF16,
                                      isOutput=False) for i in range(8)]
    wq_p = [nc.declare_dram_parameter(f"wq{i}", [128, 8 * 128], BF16,
                                      isOutput=False) for i in range(8)]
    wk_p = [nc.declare_dram_parameter(f"wk{i}", [128, 8 * 128], BF16,
                                      isOutput=False) for i in range(8)]
    msk = nc.declare_dram_parameter("msk", [128, KT], F32, isOutput=False)
    idn = nc.declare_dram_parameter("idn", [128, 128], BF16, isOutput=False)
    out = nc.declare_dram_parameter("out", [S, D], F32, isOutput=True)

    att_sched = _attnv_schedule()
    # pt buffer depth = max (exps emitted) - (chunks consumed) + margin
    need = 0
    consumed = 0
    for s in range(200):
        consumed += len(att_sched.get(s, []))
        emitted = min(s + 1, 128)
        need = max(need, emitted - consumed)
    ptp_bufs = need + 2

    with tile.TileContext(nc) as tc:
        with (
            tc.tile_pool(name="persist", bufs=1) as pers,
            tc.tile_pool(name="pt", bufs=ptp_bufs) as ptp,
            tc.tile_pool(name="outp", bufs=3) as outp,
            tc.tile_pool(name="rr", bufs=4) as rrp,
            tc.tile_pool(name="pp", bufs=2, space="PSUM") as pp,
            tc.tile_pool(name="psc", bufs=2, space="PSUM") as psc,
            tc.tile_pool(name="pod", bufs=2, space="PSUM") as pod,
        ):
            # ---------- constants (tiny, front of scalar queue) ----------
            mask_sb = pers.tile([128, KT], F32)
            nc.scalar.dma_start(out=mask_sb, in_=msk[:, :])
            id_sb = pers.tile([128, 128], BF16)
            nc.scalar.dma_start(out=id_sb, in_=idn[:, :])

            # ---------- persistent input pieces ----------
            XQ = [pers.tile([128, 8, 512], BF16, name=f"XQ{i}")
                  for i in range(2)]
            XK = [pers.tile([128, 8, 512], BF16, name=f"XK{i}")
                  for i in range(2)]
            WV = [pers.tile([128, 8, 512], BF16, name=f"WV{i}")
                  for i in range(2)]
            XV = [pers.tile([128, 8, 128], BF16, name=f"XV{i}")
                  for i in range(8)]
            WQ = [pers.tile([128, 8, 128], BF16, name=f"WQ{i}")
                  for i in range(8)]
            WK = [pers.tile([128, 8, 128], BF16, name=f"WK{i}")
                  for i in range(8)]

            def ld(dst, param, w):
                nc_eng = nc.scalar if w == 0 else nc.gpsimd
                nc_eng.dma_start(
                    out=dst, in_=param[:, :].rearrange("p (k s) -> p k s",
                                                       s=dst.shape[2]))

            junk_s = pers.tile([128, 1], BF16)
            junk_g = pers.tile([1, 2], BF16)
            warm = pers.tile([128, 1], F32)
            nc.scalar.copy(warm, mask_sb[:, 0:1])            # warm ACT clock

            # scalar queue: the whole critical prefix ungated (it must all
            # land before the first scores anyway) — many in-flight
            # descriptors are REQUIRED to saturate the DMA fabric (~400GB/s
            # needs 3+; one lone descriptor crawls at ~100GB/s).
            ld(WQ[0], wq_p[0], 0)
            ld(XQ[0], xq_p[0], 0)
            ld(WK[0], wk_p[0], 0)
            ld(XK[0], xk_p[0], 0)

            # gpsimd queue: the rest in ~2.5MB waves, one junk-read gate per
            # wave so waves complete in priority order while keeping enough
            # descriptors in flight for full bandwidth.  First gate waits on
            # the scalar prefix so these don't steal its bandwidth.
            nc.gpsimd.tensor_copy(junk_g, XK[0][0:1, 0, 0:2])
            ld(XK[1], xk_p[1], 1)          # wave 1: 2.5MB
            ld(XQ[1], xq_p[1], 1)
            ld(WQ[1], wq_p[1], 1)
            ld(WK[1], wk_p[1], 1)
            nc.gpsimd.tensor_copy(junk_g, XK[1][0:1, 0, 0:2])
            ld(WV[0], wv_p[0], 1)          # wave 2: 2MB
            for i in range(4):
                ld(XV[i], xv_p[i], 1)
            nc.gpsimd.tensor_copy(junk_g, XQ[1][0:1, 0, 0:2])
            ld(WV[1], wv_p[1], 1)          # wave 3: 2MB
            for i in range(4, 8):
                ld(XV[i], xv_p[i], 1)
            nc.gpsimd.tensor_copy(junk_g, WV[0][0:1, 0, 0:2])
            for r in (2, 3):               # wave 4: 1MB
                ld(WQ[r], wq_p[r], 1)
                ld(WK[r], wk_p[r], 1)
            nc.gpsimd.tensor_copy(junk_g, XV[7][0:1, 0, 0:2])
            for r in (4, 5):               # wave 5: 1MB
                ld(WQ[r], wq_p[r], 1)
                ld(WK[r], wk_p[r], 1)
            nc.gpsimd.tensor_copy(junk_g, WK[3][0:1, 0, 0:2])
            for r in (6, 7):               # wave 6: 1MB
                ld(WQ[r], wq_p[r], 1)
                ld(WK[r], wk_p[r], 1)

            # HAM warm-up: junk matmuls while the first pieces stream, so
            # the first projection matmuls run at 2.4 GHz
            for _ in range(4):
                jw = pp.tile([32, 128], F32, tag="pp")
                for j in range(9):
                    nc.tensor.matmul(
                        jw[:, :], lhsT=id_sb[0:32, 0:32],
                        rhs=id_sb[0:32, 0:128],
                        start=(j == 0), stop=(j == 8))

            # ---------- persistent activations ----------
            QT = pers.tile([128, KT, S], BF16)     # Q^T tiles: rows 128r+p
            KTt = pers.tile([128, KT, S], BF16)    # K^T
            VA = pers.tile([128, KT, H * 65], BF16)  # V with ones columns

            def wq_s(k, r):
                return WQ[r][:, k, :]

            def wk_s(k, r):
                return WK[r][:, k, :]

            def xq_s(k, sc):
                return XQ[sc][:, k, :]

            def xk_s(k, sc):
                return XK[sc][:, k, :]

            def xv_s(k, st):
                return XV[st][:, k, :]

            def wv_s(k, dc):
                return WV[dc][:, k, :]

            def va_slices(st, dc):
                dst = VA[:, st, :].rearrange("p (h w) -> p h w", w=65)
                return dst[:, dc * 8:(dc + 1) * 8, 0:64]

            class ProjStepper:
                """Emit projection chains one matmul at a time so they pace
                evenly between attention steps.

                Chain specs:
                  ('qk', which, sc, r)   -- 8 k-steps, full contraction
                  ('v8', st, dc)         -- 8 k-steps, V projection
                """

                def __init__(self, chains, pool, tag, max_active=1):
                    self.pending = list(chains)
                    self.active = []   # [psum_tile, chain_spec, next_j]
                    self.rr = 0
                    self.pool, self.tag = pool, tag
                    self.MAX_ACTIVE = max_active

                def _start(self):
                    if self.pending:
                        spec = self.pending.pop(0)
                        pq = self.pool.tile([128, 512], F32, tag=self.tag,
                                            name=f"pq_{'_'.join(map(str, spec))}")
                        self.active.append([pq, spec, 0])

                def step(self, n=2):
                    for _ in range(n):
                        while len(self.active) < self.MAX_ACTIVE and self.pending:
                            self._start()
                        if not self.active:
                            return
                        ent = self.active[self.rr % len(self.active)]
                        self.rr += 1
                        pq, spec, j = ent
                        if spec[0] == 'qk':
                            _, which, sc, r = spec
                            w_s, x_s = ((wq_s, xq_s) if which == 0
                                        else (wk_s, xk_s))
                            nc.tensor.matmul(
                                pq[:, :], lhsT=w_s(j, r), rhs=x_s(j, sc),
                                start=(j == 0), stop=(j == KT - 1))
                        else:
                            _, st, dc = spec
                            nc.tensor.matmul(
                                pq[:, :], lhsT=xv_s(j, st), rhs=wv_s(j, dc),
                                start=(j == 0), stop=(j == KT - 1))
                        ent[2] += 1
                        if ent[2] == KT:
                            if spec[0] == 'qk':
                                _, which, sc, r = spec
                                dstT = QT if which == 0 else KTt
                                nc.vector.tensor_copy(
                                    dstT[:, r, bass.ts(sc, 512)], pq)
                            else:
                                _, st, dc = spec
                                nc.vector.tensor_copy(
                                    va_slices(st, dc),
                                    pq[:, :].rearrange("p (h w) -> p h w",
                                                       w=64))
                                if dc == 1:
                                    ones = VA[:, st, :].rearrange(
                                        "p (h w) -> p h w", w=65)
                                    nc.vector.memset(ones[:, :, 64:65], 1.0)
                            self.active.remove(ent)

                def finish(self):
                    while self.active or self.pending:
                        self.step(1)

            # prelude: QK pair-0 sc0 chains (first scores need only these);
            # 2-active so consecutive matmuls alternate psum banks.
            pre = ProjStepper([('qk', 1, 0, 0), ('qk', 0, 0, 0)],
                              pp, "pp", max_active=2)
            pre.finish()

            # main-loop steppers
            sc1 = ProjStepper([('qk', 1, 1, 0), ('qk', 0, 1, 0)], pp, "pp")
            pair = {r: ProjStepper([('qk', w, sc, r)
                                    for w in (0, 1) for sc in (0, 1)],
                                   pp, "pp") for r in range(1, 8)}
            vstep = ProjStepper([('v8', st, dc)
                                 for st in range(8) for dc in range(2)],
                                pp, "pp")

            def pace(s):
                if s < 16:
                    pair[1].step(2)
                if 1 <= s <= 4:
                    sc1.step(4)
                if 6 <= s <= 27:
                    vstep.step(6)
                r_next = s // 16 + 1        # pair r+1 paced over pair r
                if 16 <= s and r_next <= 7:
                    pair[r_next].step(2)
                if s == 27:
                    vstep.finish()
                if s == 111:
                    for r in range(2, 8):
                        pair[r].finish()

            OPs = {}

            def scores_exp(r, qc, c):
                ps = psc.tile([128, 1024], F32, tag="psc")
                nc.tensor.matmul(
                    ps[:, 0:512],
                    lhsT=KTt[0:64, r, bass.ts(c, 128)],
                    rhs=QT[0:64, r, bass.ts(qc, 512)],
                    start=True, stop=True, tile_position=(0, 0))
                nc.tensor.matmul(
                    ps[:, 512:1024],
                    lhsT=KTt[64:128, r, bass.ts(c, 128)],
                    rhs=QT[64:128, r, bass.ts(qc, 512)],
                    start=True, stop=True, tile_position=(64, 0))
                pt = ptp.tile([128, 1024], BF16, tag="pt")
                nc.scalar.activation(pt, ps, AF.Exp,
                                     bias=mask_sb[:, c:c + 1], scale=1.0)
                return pt

            def attnv_chunk(r, po1, po2, pt, c):
                for ph, (po, hh) in enumerate(((po1, 2 * r), (po2, 2 * r + 1))):
                    for qt in range(4):
                        nc.tensor.matmul(
                            po[:, qt, :],
                            lhsT=pt[:, ph * 512 + qt * 128:
                                    ph * 512 + (qt + 1) * 128],
                            rhs=VA[:, c, hh * 65:(hh + 1) * 65],
                            start=(c == 0 and qt == 0),
                            stop=(c == KT - 1))

            def outphase(r, qc, po1, po2, tail):
                rh = r // 4
                OP = OPs.get((rh, qc))
                if OP is None:
                    OP = OPs[(rh, qc)] = outp.tile(
                        [128, 4, 512], BF16, tag="outp", name=f"OP{rh}_{qc}")
                # bulk-drain po psum fast (pod rotates every 1.5 groups),
                # then normalize on the idle gpsimd engine out of SBUF.
                sb1 = rrp.tile([128, 4, 65], F32, tag="sbp", bufs=2,
                               name=f"sb1_{r}_{qc}")
                sb2 = rrp.tile([128, 4, 65], F32, tag="sbp", bufs=2,
                               name=f"sb2_{r}_{qc}")
                nc.vector.tensor_copy(sb1, po1)
                nc.vector.tensor_copy(sb2, po2)
                rr1 = rrp.tile([128, 4, 1], F32, tag="rr", bufs=2,
                               name=f"rr1_{r}_{qc}")
                rr2 = rrp.tile([128, 4, 1], F32, tag="rr", bufs=2,
                               name=f"rr2_{r}_{qc}")
                nc.vector.reciprocal(rr1, sb1[:, :, 64:65])
                nc.vector.reciprocal(rr2, sb2[:, :, 64:65])
                for qt in range(4):
                    for ph, (sb, rr) in enumerate(((sb1, rr1), (sb2, rr2))):
                        nc.gpsimd.tensor_scalar_mul(
                            OP[:, qt, (r % 4) * 128 + ph * DH:
                               (r % 4) * 128 + (ph + 1) * DH],
                            sb[:, qt, 0:64], rr[:, qt, 0:1])
                    if tail:
                        nc.gpsimd.dma_start(
                            out=out[qc * 512 + qt * 128:
                                    qc * 512 + (qt + 1) * 128,
                                    bass.ts(rh, 512)],
                            in_=OP[:, qt, :])
                if r % 4 == 3 and not tail:
                    nc.gpsimd.dma_start(
                        out=out[bass.ts(qc, 512), bass.ts(rh, 512)].rearrange(
                            "(a p) w -> p a w", p=128),
                        in_=OP[:, :, :])
                    OPs[(rh, qc)] = None

            # ---- main loop ----
            pts_store = {}
            po_store = {}
            n_done = 0
            for s in range(300):
                if s < 128:
                    r, qc, c = s // 16, (s // 8) % 2, s % 8
                    pts_store[(s // 8, c)] = scores_exp(r, qc, c)
                    pace(s)
                for (g, cc) in att_sched.get(s, []):
                    gr, gqc = g // 2, g % 2
                    if cc == 0:
                        hp1 = pod.tile([128, 4, 65], F32, tag="pod",
                                       name=f"po1_g{g}")
                        hp2 = pod.tile([128, 4, 65], F32, tag="pod",
                                       name=f"po2_g{g}")
                        po_store[g] = (hp1, hp2)
                    hp1, hp2 = po_store[g]
                    attnv_chunk(gr, hp1, hp2, pts_store.pop((g, cc)), cc)
                    if cc == KT - 1:
                        outphase(gr, gqc, hp1, hp2, tail=(g == 15))
                        del po_store[g]
                        n_done += 1
                if s >= 128 and n_done == 16:
                    break

    _split_excess_waits(nc)
    return nc


def _retile(a, blk):
    """[D, D]-like [k*128+p, n] -> list of pieces [128, 8*blk] where piece i
    holds columns [i*blk, (i+1)*blk), layout p-major then (k, col)."""
    D0, D1 = a.shape
    out = []
    t = a.reshape(8, 128, D1)
    for i in range(D1 // blk):
        piece = t[:, :, i * blk:(i + 1) * blk].transpose(1, 0, 2)
        out.append(np.ascontiguousarray(piece.reshape(128, 8 * blk)))
    return out


def _prep_inputs(queries, keys, values, valid_lens, w_q, w_k, w_v):
    bf = ml_dtypes.bfloat16
    wq_b = (w_q.astype(np.float32) / np.sqrt(DH)).astype(bf)
    wk_b = w_k.astype(np.float32).astype(bf)
    wv_b = w_v.astype(np.float32).astype(bf)
    wq_t = _retile(wq_b, 128)
    wk_t = _retile(wk_b, 128)
    wv_t = _retile(wv_b, 512)
    idn = np.eye(128, dtype=bf)
    in_maps = []
    for b in range(B):
        mask = np.where(np.arange(S) < int(valid_lens[b]), 0.0, NEG)
        mask = np.ascontiguousarray(
            mask.reshape(KT, 128).T.astype(np.float32))          # [128, KT]
        xq_t = _retile(queries[b].astype(np.float32).T.astype(bf), 512)
        xk_t = _retile(keys[b].astype(np.float32).T.astype(bf), 512)
        xv_t = _retile(values[b].astype(np.float32).T.astype(bf), 128)
        m = dict(msk=mask, idn=idn)
        for i in range(2):
            m[f"xq{i}"] = xq_t[i]
            m[f"xk{i}"] = xk_t[i]
            m[f"wv{i}"] = wv_t[i]
        for i in range(8):
            m[f"xv{i}"] = xv_t[i]
            m[f"wq{i}"] = wq_t[i]
            m[f"wk{i}"] = wk_t[i]
        in_maps.append(m)
    return in_maps


def kernel(queries, keys, values, valid_lens, w_q, w_k, w_v, _want_results=False):
    queries = np.asarray(queries)
    keys = np.asarray(keys)
    values = np.asarray(values)
    valid_lens = np.asarray(valid_lens)
    w_q, w_k, w_v = np.asarray(w_q), np.asarray(w_k), np.asarray(w_v)
    if "nc" not in _cache:
        _cache["nc"] = _build_program()
    nc = _cache["nc"]
    in_maps = _prep_inputs(queries, keys, values, valid_lens, w_q, w_k, w_v)
    res = run_bass_kernel_spmd(nc, in_maps, list(range(N_CORES)))
    out = np.stack([res.results[b]["out"] for b in range(B)]).astype(np.float32)
    # valid_len == 0: reference softmaxes an all -1e9 row -> uniform attention.
    for b in range(B):
        if int(valid_lens[b]) == 0:
            vfull = values[b].astype(np.float32) @ w_v.astype(np.float32)
            out[b] = np.broadcast_to(vfull.mean(axis=0), (S, D))
    if _want_results:
        return out, res
    return out


# revision 13
# speedup vs baseline: 1.0757x; 1.0323x over previous
"""MultiHeadAttention Trainium2 Bass kernel (B=8, S=1024, D=1024, H=16).

Sharding: data-parallel over batch — core b computes batch element b.

v2 schedule rework. Per-core algorithm (matmuls bf16, PSUM fp32):
  * Host pre-tiles every input into block-ordered contiguous pieces so the
    DMA stream can be priority-ordered: wq_r0, xq_sc0, wk_r0, xk_sc0 (the
    4.5MB prefix that unblocks the first scores+exp at ~7us), then xk_sc1,
    xq_sc1, pair-1 weights, wv/xv (V projection inputs), then the remaining
    wq/wk r-blocks.  Pieces are issued 2-deep with junk-read gates so they
    complete in priority order at full bandwidth.
  * Projections on PE: Q^T/K^T r-pair chains (8 k-steps, [128,512] PSUM),
    V chains xv-stationary x wv-moving -> VA [kpos, d] with a ones column
    per head (softmax denominator via the attnV matmul).
  * Scores: per (r, qc, c): scores^T[kpos, q] = K_h x Q_h^T, two heads
    packed via tile_position row groups (K=64 each), PSUM [128, 1024].
  * Softmax: ONE ScalarE exp per step, -1e9 key mask fused via the
    per-partition bias port, bf16 P^T out.  The exp stream is the pacer
    (~1.11us/step); every other engine is scheduled to hide under it.
  * attnV: pt-block stationary x [V_h | ones]-moving, accumulated over c
    in PSUM [128, 4, 65] per head.  attnV chunks are scheduled by a static
    greedy table that respects exp completion, V-projection progress and
    PSUM residency; early groups are staggered, later groups catch up at
    ~1.3 chunks/step.
  * Out-phase: bulk-copy po PSUM->SBUF on DVE (frees the psum bank fast),
    reciprocal on DVE, normalization muls on the idle gpsimd engine,
    DMA out from gpsimd.
"""
import numpy as np
import ml_dtypes

import concourse.bass as bass
import concourse.mybir as mybir
import concourse.tile as tile
from concourse.bass_utils import run_bass_kernel_spmd

F32 = mybir.dt.float32
BF16 = mybir.dt.bfloat16
AF = mybir.ActivationFunctionType

B, S, D, H = 8, 1024, 1024, 16
DH = D // H          # 64
KT = 8               # contraction chunks of 128
NEG = -1.0e9
N_CORES = 8

_cache = {}


def _split_excess_waits(nc, limit: int = 1):
    """Walrus TPB instruction structs encode exactly ONE wait; hoist excess
    waits emitted by Tile into standalone InstEventSemaphore instructions."""
    ctr = 0
    for f in nc.m.functions:
        for bb in f.blocks:
            new = []
            changed = False
            for inst in bb.instructions:
                si = inst.sync_info
                waits = list(si.on_wait) if si is not None and si.on_wait else []
                if len(waits) > limit:
                    excess, keep = waits[:-limit], waits[-limit:]
                    for w in excess:
                        ctr += 1
                        new.append(mybir.InstEventSemaphore(
                            name=f"wsplit-{ctr}",
                            engine=inst.engine,
                            ins=[], outs=[],
                            sync_info=mybir.SyncInfo(on_wait=[w], on_update=[]),
                        ))
                    inst.sync_info = mybir.SyncInfo(
                        on_wait=keep,
                        on_update=list(si.on_update) if si.on_update else [],
                    )
                    changed = True
                new.append(inst)
            if changed:
                bb.instructions = new
    return ctr


def _attnv_schedule():
    """step -> list of (g, c) attnV chunk slots.  Static greedy: respects
    exp completion (chunk (g,c) after step 8g+c), V-projection progress
    (VA st=c complete ~step 6 + 16(c+1)/6), and keeps groups compact so at
    most 2 po accumulators are open (pod pool bufs=3)."""
    va_ready = [6 + (16 * (c + 1) + 5) // 6 for c in range(8)]   # st=c done
    sched = {}
    pend = [(g, c) for g in range(16) for c in range(8)]
    i = 0
    last_close = -10                       # step of the last (g, 7) chunk
    for s in range(0, 300):
        if i >= len(pend):
            break
        cap = 0
        if s >= 10:
            cap = 1
        if s >= 36 and s % 2 == 0:
            cap = 2
        if s >= 128:                       # drain steps: no scores/exp
            cap = 3
        got = []
        for _ in range(cap):
            if i >= len(pend):
                break
            g, c = pend[i]
            if 8 * g + c >= s:             # exp not done yet
                break
            if va_ready[c] > s:            # V projection not landed
                break
            # keep >=2 steps between closing group g and opening g+1: the
            # pod psum slots alias one group apart (bufs=2), so attnV(g+1)
            # c0 must wait for outphase(g)'s bulk psum->SBUF drain.
            if c == 0 and s < last_close + 2:
                break
            got.append((g, c))
            i += 1
            if c == KT - 1:
                last_close = s
                break                      # never open g+1 in the same step
        if got:
            sched[s] = got
    assert i == len(pend), f"attnV schedule incomplete: {i}/128"
    return sched


def _build_program():
    nc = bass.Bass()
    # pre-tiled inputs (bf16): see _prep_inputs for layouts
    xq_p = [nc.declare_dram_parameter(f"xq{i}", [128, 8 * 512], BF16,
                                      isOutput=False) for i in range(2)]
    xk_p = [nc.declare_dram_parameter(f"xk{i}", [128, 8 * 512], BF16,
                                      isOutput=False) for i in range(2)]
    wv_p = [nc.declare_dram_parameter(f"wv{i}", [128, 8 * 512], BF16,
                                      isOutput=False) for i in range(2)]
    xv_p = [nc.declare_dram_parameter(f"xv{i}", [128, 8 * 128], BF16,
                                      isOutput=False) for i in range(8)]
    wq_p = [nc.declare_dram_parameter(f"wq{i}", [128, 8 * 128], BF16,
                                      isOutput=False) for i in range(8)]
    wk_p = [nc.declare_dram_parameter(f"wk{i}", [128, 8 * 128], BF16,
                                      isOutput=False) for i in range(8)]
    msk = nc.declare_dram_parameter("msk", [128, KT], F32, isOutput=False)
    idn = nc.declare_dram_parameter("idn", [128, 128], BF16, isOutput=False)
    out = nc.declare_dram_parameter("out", [S, D], F32, isOutput=True)

    att_sched = _attnv_schedule()
    # pt buffer depth = max (exps emitted) - (chunks consumed) + margin
    need = 0
    consumed = 0
    for s in range(200):
        consumed += len(att_sched.get(s, []))
        emitted = min(s + 1, 128)
        need = max(need, emitted - consumed)
    ptp_bufs = need + 2

    with tile.TileContext(nc) as tc:
        with (
            tc.tile_pool(name="persist", bufs=1) as pers,
            tc.tile_pool(name="pt", bufs=ptp_bufs) as ptp,
            tc.tile_pool(name="outp", bufs=3) as outp,
            tc.tile_pool(name="rr", bufs=4) as rrp,
            tc.tile_pool(name="pp", bufs=2, space="PSUM") as pp,
            tc.tile_pool(name="psc", bufs=2, space="PSUM") as psc,
            tc.tile_pool(name="pod", bufs=2, space="PSUM") as pod,
        ):
            # ---------- constants (tiny, front of scalar queue) ----------
            mask_sb = pers.tile([128, KT], F32)
            nc.scalar.dma_start(out=mask_sb, in_=msk[:, :])
            id_sb = pers.tile([128, 128], BF16)
            nc.scalar.dma_start(out=id_sb, in_=idn[:, :])

            # ---------- persistent input pieces ----------
            XQ = [pers.tile([128, 8, 512], BF16, name=f"XQ{i}")
                  for i in range(2)]
            XK = [pers.tile([128, 8, 512], BF16, name=f"XK{i}")
                  for i in range(2)]
            WV = [pers.tile([128, 8, 512], BF16, name=f"WV{i}")
                  for i in range(2)]
            XV = [pers.tile([128, 8, 128], BF16, name=f"XV{i}")
                  for i in range(8)]
            WQ = [pers.tile([128, 8, 128], BF16, name=f"WQ{i}")
                  for i in range(8)]
            WK = [pers.tile([128, 8, 128], BF16, name=f"WK{i}")
                  for i in range(8)]

            def ld(dst, param, w):
                nc_eng = nc.scalar if w == 0 else nc.gpsimd
                nc_eng.dma_start(
                    out=dst, in_=param[:, :].rearrange("p (k s) -> p k s",
                                                       s=dst.shape[2]))

            junk_s = pers.tile([128, 1], BF16)
            junk_g = pers.tile([1, 2], BF16)
            warm = pers.tile([128, 1], F32)
            nc.scalar.copy(warm, mask_sb[:, 0:1])            # warm ACT clock

            # scalar queue: the whole critical prefix ungated (it must all
            # land before the first scores anyway) — many in-flight
            # descriptors are REQUIRED to saturate the DMA fabric (~400GB/s
            # needs 3+; one lone descriptor crawls at ~100GB/s).
            ld(WQ[0], wq_p[0], 0)
            ld(XQ[0], xq_p[0], 0)
            ld(WK[0], wk_p[0], 0)
            ld(XK[0], xk_p[0], 0)

            # gpsimd queue: the rest, priority-ordered with a sliding ~2.5MB
            # in-flight window.  CRUCIAL: a plain junk-READ gate has no data
            # dependency with the later dma_starts, so the Tile scheduler
            # hoists the DMA issues above it and the whole stream floods the
            # fabric at once, starving the critical prefix.  Instead each
            # gate WRITES two junk elements INTO the next piece's
            # destination tile (read from an earlier piece): the dma_start
            # then has a WAW dependency on the gate and cannot be hoisted.
            rest = ([(XK[1], xk_p[1], 8), (XQ[1], xq_p[1], 8),
                     (WQ[1], wq_p[1], 2), (WK[1], wk_p[1], 2),
                     (WV[0], wv_p[0], 8)]
                    + [(XV[i], xv_p[i], 2) for i in range(4)]
                    + [(WV[1], wv_p[1], 8)]
                    + [(XV[i], xv_p[i], 2) for i in range(4, 8)])
            for r in range(2, 8):
                rest.append((WQ[r], wq_p[r], 2))
                rest.append((WK[r], wk_p[r], 2))
            sizes = [kb for (_, _, kb) in rest]
            cum = [0]
            for kb in sizes:
                cum.append(cum[-1] + kb)
            for i, (dst, param, kb) in enumerate(rest):
                # gate on the latest piece j with >=20 (KB/partition) of
                # stream between j's end and this piece's start; the first
                # pieces gate on the scalar prefix (XK[0]).
                j = None
                for jj in range(i - 1, -1, -1):
                    if cum[i] - cum[jj + 1] >= 20:
                        j = jj
                        break
                src = rest[j][0] if j is not None else XK[0]
                nc.gpsimd.tensor_copy(dst[0:1, 0, 0:2], src[0:1, 0, 0:2])
                ld(dst, param, 1)

            # HAM warm-up: junk matmuls while the first pieces stream, so
            # the first projection matmuls run at 2.4 GHz
            for _ in range(4):
                jw = pp.tile([32, 128], F32, tag="pp")
                for j in range(9):
                    nc.tensor.matmul(
                        jw[:, :], lhsT=id_sb[0:32, 0:32],
                        rhs=id_sb[0:32, 0:128],
                        start=(j == 0), stop=(j == 8))

            # ---------- persistent activations ----------
            QT = pers.tile([128, KT, S], BF16)     # Q^T tiles: rows 128r+p
            KTt = pers.tile([128, KT, S], BF16)    # K^T
            VA = pers.tile([128, KT, H * 65], BF16)  # V with ones columns

            def wq_s(k, r):
                return WQ[r][:, k, :]

            def wk_s(k, r):
                return WK[r][:, k, :]

            def xq_s(k, sc):
                return XQ[sc][:, k, :]

            def xk_s(k, sc):
                return XK[sc][:, k, :]

            def xv_s(k, st):
                return XV[st][:, k, :]

            def wv_s(k, dc):
                return WV[dc][:, k, :]

            def va_slices(st, dc):
                dst = VA[:, st, :].rearrange("p (h w) -> p h w", w=65)
                return dst[:, dc * 8:(dc + 1) * 8, 0:64]

            class ProjStepper:
                """Emit projection chains one matmul at a time so they pace
                evenly between attention steps.

                Chain specs:
                  ('qk', which, sc, r)   -- 8 k-steps, full contraction
                  ('v8', st, dc)         -- 8 k-steps, V projection
                """

                def __init__(self, chains, pool, tag, max_active=1):
                    self.pending = list(chains)
                    self.active = []   # [psum_tile, chain_spec, next_j]
                    self.rr = 0
                    self.pool, self.tag = pool, tag
                    self.MAX_ACTIVE = max_active

                def _start(self):
                    if self.pending:
                        spec = self.pending.pop(0)
                        pq = self.pool.tile([128, 512], F32, tag=self.tag,
                                            name=f"pq_{'_'.join(map(str, spec))}")
                        self.active.append([pq, spec, 0])

                def step(self, n=2):
                    for _ in range(n):
                        while len(self.active) < self.MAX_ACTIVE and self.pending:
                            self._start()
                        if not self.active:
                            return
                        ent = self.active[self.rr % len(self.active)]
                        self.rr += 1
                        pq, spec, j = ent
                        if spec[0] == 'qk':
                            _, which, sc, r = spec
                            w_s, x_s = ((wq_s, xq_s) if which == 0
                                        else (wk_s, xk_s))
                            nc.tensor.matmul(
                                pq[:, :], lhsT=w_s(j, r), rhs=x_s(j, sc),
                                start=(j == 0), stop=(j == KT - 1))
                        else:
                            _, st, dc = spec
                            nc.tensor.matmul(
                                pq[:, :], lhsT=xv_s(j, st), rhs=wv_s(j, dc),
                                start=(j == 0), stop=(j == KT - 1))
                        ent[2] += 1
                        if ent[2] == KT:
                            if spec[0] == 'qk':
                                _, which, sc, r = spec
                                dstT = QT if which == 0 else KTt
                                nc.vector.tensor_copy(
                                    dstT[:, r, bass.ts(sc, 512)], pq)
                            else:
                                _, st, dc = spec
                                nc.vector.tensor_copy(
                                    va_slices(st, dc),
                                    pq[:, :].rearrange("p (h w) -> p h w",
                                                       w=64))
                                if dc == 1:
                                    ones = VA[:, st, :].rearrange(
                                        "p (h w) -> p h w", w=65)
                                    nc.vector.memset(ones[:, :, 64:65], 1.0)
                            self.active.remove(ent)

                def finish(self):
                    while self.active or self.pending:
                        self.step(1)

            # prelude: QK pair-0 sc0 chains (first scores need only these);
            # 2-active so consecutive matmuls alternate psum banks.
            pre = ProjStepper([('qk', 1, 0, 0), ('qk', 0, 0, 0)],
                              pp, "pp", max_active=2)
            pre.finish()

            # main-loop steppers
            sc1 = ProjStepper([('qk', 1, 1, 0), ('qk', 0, 1, 0)], pp, "pp")
            pair = {r: ProjStepper([('qk', w, sc, r)
                                    for w in (0, 1) for sc in (0, 1)],
                                   pp, "pp") for r in range(1, 8)}
            vstep = ProjStepper([('v8', st, dc)
                                 for st in range(8) for dc in range(2)],
                                pp, "pp")

            def pace(s):
                if s < 16:
                    pair[1].step(2)
                if 1 <= s <= 4:
                    sc1.step(4)
                if 6 <= s <= 27:
                    vstep.step(6)
                r_next = s // 16 + 1        # pair r+1 paced over pair r
                if 16 <= s and r_next <= 7:
                    pair[r_next].step(2)
                if s == 27:
                    vstep.finish()
                if s == 111:
                    for r in range(2, 8):
                        pair[r].finish()

            OPs = {}

            def scores_exp(r, qc, c):
                ps = psc.tile([128, 1024], F32, tag="psc")
                nc.tensor.matmul(
                    ps[:, 0:512],
                    lhsT=KTt[0:64, r, bass.ts(c, 128)],
                    rhs=QT[0:64, r, bass.ts(qc, 512)],
                    start=True, stop=True, tile_position=(0, 0))
                nc.tensor.matmul(
                    ps[:, 512:1024],
                    lhsT=KTt[64:128, r, bass.ts(c, 128)],
                    rhs=QT[64:128, r, bass.ts(qc, 512)],
                    start=True, stop=True, tile_position=(64, 0))
                pt = ptp.tile([128, 1024], BF16, tag="pt")
                nc.scalar.activation(pt, ps, AF.Exp,
                                     bias=mask_sb[:, c:c + 1], scale=1.0)
                return pt

            def attnv_chunk(r, po1, po2, pt, c):
                for ph, (po, hh) in enumerate(((po1, 2 * r), (po2, 2 * r + 1))):
                    for qt in range(4):
                        nc.tensor.matmul(
                            po[:, qt, :],
                            lhsT=pt[:, ph * 512 + qt * 128:
                                    ph * 512 + (qt + 1) * 128],
                            rhs=VA[:, c, hh * 65:(hh + 1) * 65],
                            start=(c == 0 and qt == 0),
                            stop=(c == KT - 1))

            def outphase(r, qc, po1, po2, tail):
                rh = r // 4
                OP = OPs.get((rh, qc))
                if OP is None:
                    OP = OPs[(rh, qc)] = outp.tile(
                        [128, 4, 512], BF16, tag="outp", name=f"OP{rh}_{qc}")
                # bulk-drain po psum fast (pod rotates every 1.5 groups),
                # then normalize on the idle gpsimd engine out of SBUF.
                sb1 = rrp.tile([128, 4, 65], F32, tag="sbp", bufs=2,
                               name=f"sb1_{r}_{qc}")
                sb2 = rrp.tile([128, 4, 65], F32, tag="sbp", bufs=2,
                               name=f"sb2_{r}_{qc}")
                nc.vector.tensor_copy(sb1, po1)
                nc.vector.tensor_copy(sb2, po2)
                rr1 = rrp.tile([128, 4, 1], F32, tag="rr", bufs=2,
                               name=f"rr1_{r}_{qc}")
                rr2 = rrp.tile([128, 4, 1], F32, tag="rr", bufs=2,
                               name=f"rr2_{r}_{qc}")
                nc.vector.reciprocal(rr1, sb1[:, :, 64:65])
                nc.vector.reciprocal(rr2, sb2[:, :, 64:65])
                for qt in range(4):
                    for ph, (sb, rr) in enumerate(((sb1, rr1), (sb2, rr2))):
                        nc.gpsimd.tensor_scalar_mul(
                            OP[:, qt, (r % 4) * 128 + ph * DH:
                               (r % 4) * 128 + (ph + 1) * DH],
                            sb[:, qt, 0:64], rr[:, qt, 0:1])
                    if tail:
                        nc.gpsimd.dma_start(
                            out=out[qc * 512 + qt * 128:
                                    qc * 512 + (qt + 1) * 128,
                                    bass.ts(rh, 512)],
                            in_=OP[:, qt, :])
                if r % 4 == 3 and not tail:
                    nc.gpsimd.dma_start(
                        out=out[bass.ts(qc, 512), bass.ts(rh, 512)].rearrange(
                            "(a p) w -> p a w", p=128),
                        in_=OP[:, :, :])
                    OPs[(rh, qc)] = None

            # ---- main loop ----
            pts_store = {}
            po_store = {}
            n_done = 0
            for s in range(300):
                if s < 128:
                    r, qc, c = s // 16, (s // 8) % 2, s % 8
                    pts_store[(s // 8, c)] = scores_exp(r, qc, c)
                    pace(s)
                for (g, cc) in att_sched.get(s, []):
                    gr, gqc = g // 2, g % 2
                    if cc == 0:
                        hp1 = pod.tile([128, 4, 65], F32, tag="pod",
                                       name=f"po1_g{g}")
                        hp2 = pod.tile([128, 4, 65], F32, tag="pod",
                                       name=f"po2_g{g}")
                        po_store[g] = (hp1, hp2)
                    hp1, hp2 = po_store[g]
                    attnv_chunk(gr, hp1, hp2, pts_store.pop((g, cc)), cc)
                    if cc == KT - 1:
                        outphase(gr, gqc, hp1, hp2, tail=(g == 15))
                        del po_store[g]
                        n_done += 1
                if s >= 128 and n_done == 16:
                    break

    _split_excess_waits(nc)
    return nc


def _retile(a, blk):
    """[D, D]-like [k*128+p, n] -> list of pieces [128, 8*blk] where piece i
    holds columns [i*blk, (i+1)*blk), layout p-major then (k, col)."""
    D0, D1 = a.shape
    out = []
    t = a.reshape(8, 128, D1)
    for i in range(D1 // blk):
        piece = t[:, :, i * blk:(i + 1) * blk].transpose(1, 0, 2)
        out.append(np.ascontiguousarray(piece.reshape(128, 8 * blk)))
    return out


def _prep_inputs(queries, keys, values, valid_lens, w_q, w_k, w_v):
    bf = ml_dtypes.bfloat16
    wq_b = (w_q.astype(np.float32) / np.sqrt(DH)).astype(bf)
    wk_b = w_k.astype(np.float32).astype(bf)
    wv_b = w_v.astype(np.float32).astype(bf)
    wq_t = _retile(wq_b, 128)
    wk_t = _retile(wk_b, 128)
    wv_t = _retile(wv_b, 512)
    idn = np.eye(128, dtype=bf)
    in_maps = []
    for b in range(B):
        mask = np.where(np.arange(S) < int(valid_lens[b]), 0.0, NEG)
        mask = np.ascontiguousarray(
            mask.reshape(KT, 128).T.astype(np.float32))          # [128, KT]
        xq_t = _retile(queries[b].astype(np.float32).T.astype(bf), 512)
        xk_t = _retile(keys[b].astype(np.float32).T.astype(bf), 512)
        xv_t = _retile(values[b].astype(np.float32).T.astype(bf), 128)
        m = dict(msk=mask, idn=idn)
        for i in range(2):
            m[f"xq{i}"] = xq_t[i]
            m[f"xk{i}"] = xk_t[i]
            m[f"wv{i}"] = wv_t[i]
        for i in range(8):
            m[f"xv{i}"] = xv_t[i]
            m[f"wq{i}"] = wq_t[i]
            m[f"wk{i}"] = wk_t[i]
        in_maps.append(m)
    return in_maps


def kernel(queries, keys, values, valid_lens, w_q, w_k, w_v, _want_results=False):
    queries = np.asarray(queries)
    keys = np.asarray(keys)
    values = np.asarray(values)
    valid_lens = np.asarray(valid_lens)
    w_q, w_k, w_v = np.asarray(w_q), np.asarray(w_k), np.asarray(w_v)
    if "nc" not in _cache:
        _cache["nc"] = _build_program()
    nc = _cache["nc"]
    in_maps = _prep_inputs(queries, keys, values, valid_lens, w_q, w_k, w_v)
    res = run_bass_kernel_spmd(nc, in_maps, list(range(N_CORES)))
    out = np.stack([res.results[b]["out"] for b in range(B)]).astype(np.float32)
    # valid_len == 0: reference softmaxes an all -1e9 row -> uniform attention.
    for b in range(B):
        if int(valid_lens[b]) == 0:
            vfull = values[b].astype(np.float32) @ w_v.astype(np.float32)
            out[b] = np.broadcast_to(vfull.mean(axis=0), (S, D))
    if _want_results:
        return out, res
    return out


# revision 17
# speedup vs baseline: 1.0875x; 1.0110x over previous
"""MultiHeadAttention Trainium2 Bass kernel (B=8, S=1024, D=1024, H=16).

Sharding: data-parallel over batch — core b computes batch element b.

v2 schedule rework. Per-core algorithm (matmuls bf16, PSUM fp32):
  * Host pre-tiles every input into block-ordered contiguous pieces so the
    DMA stream can be priority-ordered: wq_r0, xq_sc0, wk_r0, xk_sc0 (the
    4.5MB prefix that unblocks the first scores+exp at ~7us), then xk_sc1,
    xq_sc1, pair-1 weights, wv/xv (V projection inputs), then the remaining
    wq/wk r-blocks.  Pieces are issued 2-deep with junk-read gates so they
    complete in priority order at full bandwidth.
  * Projections on PE: Q^T/K^T r-pair chains (8 k-steps, [128,512] PSUM),
    V chains xv-stationary x wv-moving -> VA [kpos, d] with a ones column
    per head (softmax denominator via the attnV matmul).
  * Scores: per (r, qc, c): scores^T[kpos, q] = K_h x Q_h^T, two heads
    packed via tile_position row groups (K=64 each), PSUM [128, 1024].
  * Softmax: ONE ScalarE exp per step, -1e9 key mask fused via the
    per-partition bias port, bf16 P^T out.  The exp stream is the pacer
    (~1.11us/step); every other engine is scheduled to hide under it.
  * attnV: pt-block stationary x [V_h | ones]-moving, accumulated over c
    in PSUM [128, 4, 65] per head.  attnV chunks are scheduled by a static
    greedy table that respects exp completion, V-projection progress and
    PSUM residency; early groups are staggered, later groups catch up at
    ~1.3 chunks/step.
  * Out-phase: bulk-copy po PSUM->SBUF on DVE (frees the psum bank fast),
    reciprocal on DVE, normalization muls on the idle gpsimd engine,
    DMA out from gpsimd.
"""
import numpy as np
import ml_dtypes

import concourse.bass as bass
import concourse.mybir as mybir
import concourse.tile as tile
from concourse.bass_utils import run_bass_kernel_spmd

F32 = mybir.dt.float32
BF16 = mybir.dt.bfloat16
AF = mybir.ActivationFunctionType

B, S, D, H = 8, 1024, 1024, 16
DH = D // H          # 64
KT = 8               # contraction chunks of 128
NEG = -1.0e9
N_CORES = 8

_cache = {}


def _split_excess_waits(nc, limit: int = 1):
    """Walrus TPB instruction structs encode exactly ONE wait; hoist excess
    waits emitted by Tile into standalone InstEventSemaphore instructions."""
    ctr = 0
    for f in nc.m.functions:
        for bb in f.blocks:
            new = []
            changed = False
            for inst in bb.instructions:
                si = inst.sync_info
                waits = list(si.on_wait) if si is not None and si.on_wait else []
                if len(waits) > limit:
                    excess, keep = waits[:-limit], waits[-limit:]
                    for w in excess:
                        ctr += 1
                        new.append(mybir.InstEventSemaphore(
                            name=f"wsplit-{ctr}",
                            engine=inst.engine,
                            ins=[], outs=[],
                            sync_info=mybir.SyncInfo(on_wait=[w], on_update=[]),
                        ))
                    inst.sync_info = mybir.SyncInfo(
                        on_wait=keep,
                        on_update=list(si.on_update) if si.on_update else [],
                    )
                    changed = True
                new.append(inst)
            if changed:
                bb.instructions = new
    return ctr


def _attnv_schedule():
    """step -> list of (g, c) attnV chunk slots.  Static greedy: respects
    exp completion (chunk (g,c) after step 8g+c), V-projection progress
    (VA st=c complete ~step 6 + 16(c+1)/6), and keeps groups compact so at
    most 2 po accumulators are open (pod pool bufs=3)."""
    # V chains run dc-major from step 8 at 6 MM/step: st=c's dc1 chain
    # (the later one) completes around step 19 + 4c/3.
    va_ready = [20 + (4 * c) // 3 for c in range(8)]             # st=c done
    sched = {}
    pend = [(g, c) for g in range(16) for c in range(8)]
    i = 0
    last_close = -10                       # step of the last (g, 7) chunk
    for s in range(0, 300):
        if i >= len(pend):
            break
        cap = 0
        if s >= 10:
            cap = 1
        if s >= 36 and s % 2 == 0:
            cap = 2
        if s >= 128:                       # drain steps: no scores/exp
            cap = 3
        got = []
        for _ in range(cap):
            if i >= len(pend):
                break
            g, c = pend[i]
            if 8 * g + c >= s:             # exp not done yet
                break
            if va_ready[c] > s:            # V projection not landed
                break
            # keep >=2 steps between closing group g and opening g+1: the
            # pod psum slots alias one group apart (bufs=2), so attnV(g+1)
            # c0 must wait for outphase(g)'s bulk psum->SBUF drain.
            if c == 0 and s < last_close + 2:
                break
            got.append((g, c))
            i += 1
            if c == KT - 1:
                last_close = s
                break                      # never open g+1 in the same step
        if got:
            sched[s] = got
    assert i == len(pend), f"attnV schedule incomplete: {i}/128"
    return sched


def _build_program():
    nc = bass.Bass()
    # pre-tiled inputs (bf16): see _prep_inputs for layouts
    xq_p = [nc.declare_dram_parameter(f"xq{i}", [128, 8 * 512], BF16,
                                      isOutput=False) for i in range(2)]
    xk_p = [nc.declare_dram_parameter(f"xk{i}", [128, 8 * 512], BF16,
                                      isOutput=False) for i in range(2)]
    wv_p = [nc.declare_dram_parameter(f"wv{i}", [128, 8 * 512], BF16,
                                      isOutput=False) for i in range(2)]
    xv_p = [nc.declare_dram_parameter(f"xv{i}", [128, 8 * 128], BF16,
                                      isOutput=False) for i in range(8)]
    wq_p = [nc.declare_dram_parameter(f"wq{i}", [128, 8 * 128], BF16,
                                      isOutput=False) for i in range(8)]
    wk_p = [nc.declare_dram_parameter(f"wk{i}", [128, 8 * 128], BF16,
                                      isOutput=False) for i in range(8)]
    msk = nc.declare_dram_parameter("msk", [128, KT], F32, isOutput=False)
    idn = nc.declare_dram_parameter("idn", [128, 128], BF16, isOutput=False)
    out = nc.declare_dram_parameter("out", [S, D], F32, isOutput=True)

    att_sched = _attnv_schedule()
    # pt buffer depth = max (exps emitted) - (chunks consumed) + margin
    need = 0
    consumed = 0
    for s in range(200):
        consumed += len(att_sched.get(s, []))
        emitted = min(s + 1, 128)
        need = max(need, emitted - consumed)
    ptp_bufs = need + 1

    with tile.TileContext(nc) as tc:
        with (
            tc.tile_pool(name="persist", bufs=1) as pers,
            tc.tile_pool(name="pt", bufs=ptp_bufs) as ptp,
            tc.tile_pool(name="outp", bufs=3) as outp,
            tc.tile_pool(name="rr", bufs=4) as rrp,
            tc.tile_pool(name="pp", bufs=2, space="PSUM") as pp,
            tc.tile_pool(name="psc", bufs=2, space="PSUM") as psc,
            tc.tile_pool(name="pod", bufs=2, space="PSUM") as pod,
        ):
            # ---------- constants (tiny, front of scalar queue) ----------
            mask_sb = pers.tile([128, KT], F32)
            nc.scalar.dma_start(out=mask_sb, in_=msk[:, :])
            id_sb = pers.tile([128, 128], BF16)
            nc.scalar.dma_start(out=id_sb, in_=idn[:, :])

            # ---------- persistent input pieces ----------
            XQ = [pers.tile([128, 8, 512], BF16, name=f"XQ{i}")
                  for i in range(2)]
            XK = [pers.tile([128, 8, 512], BF16, name=f"XK{i}")
                  for i in range(2)]
            WV = [pers.tile([128, 8, 512], BF16, name=f"WV{i}")
                  for i in range(2)]
            XV = [pers.tile([128, 8, 128], BF16, name=f"XV{i}")
                  for i in range(8)]
            WQ = [pers.tile([128, 8, 128], BF16, name=f"WQ{i}")
                  for i in range(8)]
            WK = [pers.tile([128, 8, 128], BF16, name=f"WK{i}")
                  for i in range(8)]

            def ld(dst, param, w):
                nc_eng = nc.scalar if w == 0 else nc.gpsimd
                nc_eng.dma_start(
                    out=dst, in_=param[:, :].rearrange("p (k s) -> p k s",
                                                       s=dst.shape[2]))

            junk_s = pers.tile([128, 1], BF16)
            junk_g = pers.tile([1, 2], BF16)
            warm = pers.tile([128, 1], F32)
            nc.scalar.copy(warm, mask_sb[:, 0:1])            # warm ACT clock

            # scalar queue: the whole critical prefix ungated (it must all
            # land before the first scores anyway) — many in-flight
            # descriptors are REQUIRED to saturate the DMA fabric (~400GB/s
            # needs 3+; one lone descriptor crawls at ~100GB/s).
            ld(WQ[0], wq_p[0], 0)
            ld(XQ[0], xq_p[0], 0)
            ld(WK[0], wk_p[0], 0)
            ld(XK[0], xk_p[0], 0)

            # gpsimd queue: the rest, priority-ordered with a sliding ~2.5MB
            # in-flight window.  CRUCIAL: a plain junk-READ gate has no data
            # dependency with the later dma_starts, so the Tile scheduler
            # hoists the DMA issues above it and the whole stream floods the
            # fabric at once, starving the critical prefix.  Instead each
            # gate WRITES two junk elements INTO the next piece's
            # destination tile (read from an earlier piece): the dma_start
            # then has a WAW dependency on the gate and cannot be hoisted.
            rest = ([(XK[1], xk_p[1], 8), (XQ[1], xq_p[1], 8),
                     (WQ[1], wq_p[1], 2), (WK[1], wk_p[1], 2),
                     (WV[0], wv_p[0], 8)]
                    + [(XV[i], xv_p[i], 2) for i in range(4)]
                    + [(WV[1], wv_p[1], 8)]
                    + [(XV[i], xv_p[i], 2) for i in range(4, 8)])
            sizes = [kb for (_, _, kb) in rest]
            cum = [0]
            for kb in sizes:
                cum.append(cum[-1] + kb)
            for i, (dst, param, kb) in enumerate(rest):
                # gate on the latest piece j with >=20 (KB/partition) of
                # stream between j's end and this piece's start; the first
                # pieces gate on the scalar prefix (XK[0]).
                j = None
                for jj in range(i - 1, -1, -1):
                    if cum[i] - cum[jj + 1] >= 20:
                        j = jj
                        break
                src = rest[j][0] if j is not None else XK[0]
                nc.gpsimd.tensor_copy(dst[0:1, 0, 0:2], src[0:1, 0, 0:2])
                ld(dst, param, 1)
            # small wq/wk tail: flood after one gate — needed only from
            # step ~32 onward, and re-gating it into the dense compute
            # phase starves it down to a trickle.
            for r in range(2, 8):
                for WX, wx_p in ((WQ, wq_p), (WK, wk_p)):
                    nc.gpsimd.tensor_copy(WX[r][0:1, 0, 0:2],
                                          XV[7][0:1, 0, 0:2])
                    ld(WX[r], wx_p[r], 1)

            # HAM warm-up: junk matmuls while the first pieces stream, so
            # the first projection matmuls run at 2.4 GHz
            for _ in range(4):
                jw = pp.tile([32, 128], F32, tag="pp")
                for j in range(9):
                    nc.tensor.matmul(
                        jw[:, :], lhsT=id_sb[0:32, 0:32],
                        rhs=id_sb[0:32, 0:128],
                        start=(j == 0), stop=(j == 8))

            # ---------- persistent activations ----------
            QT = pers.tile([128, KT, S], BF16)     # Q^T tiles: rows 128r+p
            KTt = pers.tile([128, KT, S], BF16)    # K^T
            VA = pers.tile([128, KT, H * 65], BF16)  # V with ones columns

            def wq_s(k, r):
                return WQ[r][:, k, :]

            def wk_s(k, r):
                return WK[r][:, k, :]

            def xq_s(k, sc):
                return XQ[sc][:, k, :]

            def xk_s(k, sc):
                return XK[sc][:, k, :]

            def xv_s(k, st):
                return XV[st][:, k, :]

            def wv_s(k, dc):
                return WV[dc][:, k, :]

            def va_slices(st, dc):
                dst = VA[:, st, :].rearrange("p (h w) -> p h w", w=65)
                return dst[:, dc * 8:(dc + 1) * 8, 0:64]

            class ProjStepper:
                """Emit projection chains one matmul at a time so they pace
                evenly between attention steps.

                Chain specs:
                  ('qk', which, sc, r)   -- 8 k-steps, full contraction
                  ('v8', st, dc)         -- 8 k-steps, V projection
                """

                def __init__(self, chains, pool, tag, max_active=1):
                    self.pending = list(chains)
                    self.active = []   # [psum_tile, chain_spec, next_j]
                    self.rr = 0
                    self.pool, self.tag = pool, tag
                    self.MAX_ACTIVE = max_active

                def _start(self):
                    if self.pending:
                        spec = self.pending.pop(0)
                        pq = self.pool.tile([128, 512], F32, tag=self.tag,
                                            name=f"pq_{'_'.join(map(str, spec))}")
                        self.active.append([pq, spec, 0])

                def step(self, n=2):
                    for _ in range(n):
                        while len(self.active) < self.MAX_ACTIVE and self.pending:
                            self._start()
                        if not self.active:
                            return
                        ent = self.active[self.rr % len(self.active)]
                        self.rr += 1
                        pq, spec, j = ent
                        if spec[0] == 'qk':
                            _, which, sc, r = spec
                            w_s, x_s = ((wq_s, xq_s) if which == 0
                                        else (wk_s, xk_s))
                            nc.tensor.matmul(
                                pq[:, :], lhsT=w_s(j, r), rhs=x_s(j, sc),
                                start=(j == 0), stop=(j == KT - 1))
                        else:
                            _, st, dc = spec
                            nc.tensor.matmul(
                                pq[:, :], lhsT=xv_s(j, st), rhs=wv_s(j, dc),
                                start=(j == 0), stop=(j == KT - 1))
                        ent[2] += 1
                        if ent[2] == KT:
                            if spec[0] == 'qk':
                                _, which, sc, r = spec
                                dstT = QT if which == 0 else KTt
                                nc.vector.tensor_copy(
                                    dstT[:, r, bass.ts(sc, 512)], pq)
                            else:
                                _, st, dc = spec
                                nc.vector.tensor_copy(
                                    va_slices(st, dc),
                                    pq[:, :].rearrange("p (h w) -> p h w",
                                                       w=64))
                                if dc == 1:
                                    ones = VA[:, st, :].rearrange(
                                        "p (h w) -> p h w", w=65)
                                    nc.vector.memset(ones[:, :, 64:65], 1.0)
                            self.active.remove(ent)

                def finish(self):
                    while self.active or self.pending:
                        self.step(1)

            # prelude: QK pair-0 sc0 chains (first scores need only these);
            # 2-active so consecutive matmuls alternate psum banks.
            pre = ProjStepper([('qk', 1, 0, 0), ('qk', 0, 0, 0)],
                              pp, "pp", max_active=2)
            pre.finish()

            # main-loop steppers
            sc1 = ProjStepper([('qk', 1, 1, 0), ('qk', 0, 1, 0)], pp, "pp")
            pair = {r: ProjStepper([('qk', w, sc, r)
                                    for w in (0, 1) for sc in (0, 1)],
                                   pp, "pp") for r in range(1, 8)}
            # dc-major: all dc0 chains (need only wv0+xv) before dc1, so
            # pacing never runs ahead of the wv1 DMA.
            vstep = ProjStepper([('v8', st, dc)
                                 for dc in range(2) for st in range(8)],
                                pp, "pp")

            def pace(s):
                if 1 <= s <= 4:
                    sc1.step(4)
                if 4 <= s <= 14:            # pair-1 data lands ~step 1-2
                    pair[1].step(3)
                if 8 <= s <= 29:
                    vstep.step(6)
                r_next = s // 16 + 1        # pair r+1 paced over pair r
                if 16 <= s and r_next <= 7:
                    pair[r_next].step(2)
                if s == 29:
                    vstep.finish()
                if s == 111:
                    for r in range(2, 8):
                        pair[r].finish()

            OPs = {}

            def scores_exp(r, qc, c):
                ps = psc.tile([128, 1024], F32, tag="psc")
                nc.tensor.matmul(
                    ps[:, 0:512],
                    lhsT=KTt[0:64, r, bass.ts(c, 128)],
                    rhs=QT[0:64, r, bass.ts(qc, 512)],
                    start=True, stop=True, tile_position=(0, 0))
                nc.tensor.matmul(
                    ps[:, 512:1024],
                    lhsT=KTt[64:128, r, bass.ts(c, 128)],
                    rhs=QT[64:128, r, bass.ts(qc, 512)],
                    start=True, stop=True, tile_position=(64, 0))
                pt = ptp.tile([128, 1024], BF16, tag="pt")
                nc.scalar.activation(pt, ps, AF.Exp,
                                     bias=mask_sb[:, c:c + 1], scale=1.0)
                return pt

            def attnv_chunk(r, po1, po2, pt, c):
                for ph, (po, hh) in enumerate(((po1, 2 * r), (po2, 2 * r + 1))):
                    for qt in range(4):
                        nc.tensor.matmul(
                            po[:, qt, :],
                            lhsT=pt[:, ph * 512 + qt * 128:
                                    ph * 512 + (qt + 1) * 128],
                            rhs=VA[:, c, hh * 65:(hh + 1) * 65],
                            start=(c == 0 and qt == 0),
                            stop=(c == KT - 1))

            def outphase(r, qc, po1, po2, tail):
                rh = r // 4
                OP = OPs.get((rh, qc))
                if OP is None:
                    OP = OPs[(rh, qc)] = outp.tile(
                        [128, 4, 512], BF16, tag="outp", name=f"OP{rh}_{qc}")
                # bulk-drain po psum fast (pod rotates every 1.5 groups),
                # then normalize on the idle gpsimd engine out of SBUF.
                sb1 = rrp.tile([128, 4, 65], F32, tag="sbp", bufs=2,
                               name=f"sb1_{r}_{qc}")
                sb2 = rrp.tile([128, 4, 65], F32, tag="sbp", bufs=2,
                               name=f"sb2_{r}_{qc}")
                nc.vector.tensor_copy(sb1, po1)
                nc.vector.tensor_copy(sb2, po2)
                rr1 = rrp.tile([128, 4, 1], F32, tag="rr", bufs=2,
                               name=f"rr1_{r}_{qc}")
                rr2 = rrp.tile([128, 4, 1], F32, tag="rr", bufs=2,
                               name=f"rr2_{r}_{qc}")
                nc.vector.reciprocal(rr1, sb1[:, :, 64:65])
                nc.vector.reciprocal(rr2, sb2[:, :, 64:65])
                for qt in range(4):
                    for ph, (sb, rr) in enumerate(((sb1, rr1), (sb2, rr2))):
                        nc.gpsimd.tensor_scalar_mul(
                            OP[:, qt, (r % 4) * 128 + ph * DH:
                               (r % 4) * 128 + (ph + 1) * DH],
                            sb[:, qt, 0:64], rr[:, qt, 0:1])
                    if tail:
                        nc.gpsimd.dma_start(
                            out=out[qc * 512 + qt * 128:
                                    qc * 512 + (qt + 1) * 128,
                                    bass.ts(rh, 512)],
                            in_=OP[:, qt, :])
                if r % 4 == 3 and not tail:
                    nc.gpsimd.dma_start(
                        out=out[bass.ts(qc, 512), bass.ts(rh, 512)].rearrange(
                            "(a p) w -> p a w", p=128),
                        in_=OP[:, :, :])
                    OPs[(rh, qc)] = None

            # ---- main loop ----
            pts_store = {}
            po_store = {}
            n_done = 0
            for s in range(300):
                if s < 128:
                    r, qc, c = s // 16, (s // 8) % 2, s % 8
                    pts_store[(s // 8, c)] = scores_exp(r, qc, c)
                    pace(s)
                for (g, cc) in att_sched.get(s, []):
                    gr, gqc = g // 2, g % 2
                    if cc == 0:
                        hp1 = pod.tile([128, 4, 65], F32, tag="pod",
                                       name=f"po1_g{g}")
                        hp2 = pod.tile([128, 4, 65], F32, tag="pod",
                                       name=f"po2_g{g}")
                        po_store[g] = (hp1, hp2)
                    hp1, hp2 = po_store[g]
                    attnv_chunk(gr, hp1, hp2, pts_store.pop((g, cc)), cc)
                    if cc == KT - 1:
                        outphase(gr, gqc, hp1, hp2, tail=(g == 15))
                        del po_store[g]
                        n_done += 1
                if s >= 128 and n_done == 16:
                    break

    _split_excess_waits(nc)
    return nc


def _retile(a, blk):
    """[D, D]-like [k*128+p, n] -> list of pieces [128, 8*blk] where piece i
    holds columns [i*blk, (i+1)*blk), layout p-major then (k, col)."""
    D0, D1 = a.shape
    out = []
    t = a.reshape(8, 128, D1)
    for i in range(D1 // blk):
        piece = t[:, :, i * blk:(i + 1) * blk].transpose(1, 0, 2)
        out.append(np.ascontiguousarray(piece.reshape(128, 8 * blk)))
    return out


def _prep_inputs(queries, keys, values, valid_lens, w_q, w_k, w_v):
    bf = ml_dtypes.bfloat16
    wq_b = (w_q.astype(np.float32) / np.sqrt(DH)).astype(bf)
    wk_b = w_k.astype(np.float32).astype(bf)
    wv_b = w_v.astype(np.float32).astype(bf)
    wq_t = _retile(wq_b, 128)
    wk_t = _retile(wk_b, 128)
    wv_t = _retile(wv_b, 512)
    idn = np.eye(128, dtype=bf)
    in_maps = []
    for b in range(B):
        mask = np.where(np.arange(S) < int(valid_lens[b]), 0.0, NEG)
        mask = np.ascontiguousarray(
            mask.reshape(KT, 128).T.astype(np.float32))          # [128, KT]
        xq_t = _retile(queries[b].astype(np.float32).T.astype(bf), 512)
        xk_t = _retile(keys[b].astype(np.float32).T.astype(bf), 512)
        xv_t = _retile(values[b].astype(np.float32).T.astype(bf), 128)
        m = dict(msk=mask, idn=idn)
        for i in range(2):
            m[f"xq{i}"] = xq_t[i]
            m[f"xk{i}"] = xk_t[i]
            m[f"wv{i}"] = wv_t[i]
        for i in range(8):
            m[f"xv{i}"] = xv_t[i]
            m[f"wq{i}"] = wq_t[i]
            m[f"wk{i}"] = wk_t[i]
        in_maps.append(m)
    return in_maps


def kernel(queries, keys, values, valid_lens, w_q, w_k, w_v, _want_results=False):
    queries = np.asarray(queries)
    keys = np.asarray(keys)
    values = np.asarray(values)
    valid_lens = np.asarray(valid_lens)
    w_q, w_k, w_v = np.asarray(w_q), np.asarray(w_k), np.asarray(w_v)
    if "nc" not in _cache:
        _cache["nc"] = _build_program()
    nc = _cache["nc"]
    in_maps = _prep_inputs(queries, keys, values, valid_lens, w_q, w_k, w_v)
    res = run_bass_kernel_spmd(nc, in_maps, list(range(N_CORES)))
    out = np.stack([res.results[b]["out"] for b in range(B)]).astype(np.float32)
    # valid_len == 0: reference softmaxes an all -1e9 row -> uniform attention.
    for b in range(B):
        if int(valid_lens[b]) == 0:
            vfull = values[b].astype(np.float32) @ w_v.astype(np.float32)
            out[b] = np.broadcast_to(vfull.mean(axis=0), (S, D))
    if _want_results:
        return out, res
    return out


# revision 20
# speedup vs baseline: 1.1421x; 1.0501x over previous
"""MultiHeadAttention Trainium2 Bass kernel (B=8, S=1024, D=1024, H=16).

Sharding: data-parallel over batch — core b computes batch element b.

Per-core algorithm (all matmul inputs bf16, PSUM accumulation fp32):
  * Host prep: upload X_q^T, X_k^T, X_v^T (transposed activations), W_q/8,
    W_k, W_v — all bf16 — plus an additive key mask [128, 8] fp32 and a
    bf16 identity matrix (warm-up junk matmuls only).
  * Projections on PE: Q^T = (W_q/8)^T-stationary x X_q^T-moving -> [D, S];
    K^T likewise; V = X_v^T-stationary x W_v-moving -> [S, D] stored with a
    ones-column appended per head (V_aug[:, h*65+64] = 1).  V chains are
    split into two half-contractions (k 0-3 / 4-7) so PSUM banks release
    quickly while xv/wv still stream in; halves are summed on DVE.
  * Scores: per head-pair, kpos-chunk c, q-chunk qc: scores^T[kpos, q] =
    K_h-stationary x Q_h^T-moving, two heads packed in the PE array via
    tile_position row groups (K=64 each). PSUM fp32 [128, 1024].
  * Softmax: ONE ScalarE exp per chunk, additive -1e9 mask fused via the
    per-partition bias port; no max-subtraction (scores ~ N(0,1)); output
    bf16 P^T directly to SBUF.
  * attnV, pt-STATIONARY: po[q, dh] += (P^T block)^T-stationary x
    [V_h | ones]-moving, accumulated over c in PSUM ([128, 4, 65] per
    head). Column 64 = sum of exp (softmax denominator). Output lands
    directly in [q, d] orientation — no PE transposes needed at all.
  * Out-phase: reciprocal of the denominator column, per-partition
    tensor_scalar multiply into OP [128, 4, 128] fp32, DMA straight to
    the final [S, D] layout.
"""
import numpy as np
import ml_dtypes

import concourse.bass as bass
import concourse.mybir as mybir
import concourse.tile as tile
from concourse.bass_utils import run_bass_kernel_spmd

F32 = mybir.dt.float32
BF16 = mybir.dt.bfloat16
AF = mybir.ActivationFunctionType

B, S, D, H = 8, 1024, 1024, 16
DH = D // H          # 64
KT = 8               # contraction chunks of 128
NEG = -1.0e9
N_CORES = 8

_cache = {}


def _split_excess_waits(nc, limit: int = 1):
    """Walrus TPB instruction structs encode exactly ONE wait; hoist excess
    waits emitted by Tile into standalone InstEventSemaphore instructions."""
    ctr = 0
    for f in nc.m.functions:
        for bb in f.blocks:
            new = []
            changed = False
            for inst in bb.instructions:
                si = inst.sync_info
                waits = list(si.on_wait) if si is not None and si.on_wait else []
                if len(waits) > limit:
                    excess, keep = waits[:-limit], waits[-limit:]
                    for w in excess:
                        ctr += 1
                        new.append(mybir.InstEventSemaphore(
                            name=f"wsplit-{ctr}",
                            engine=inst.engine,
                            ins=[], outs=[],
                            sync_info=mybir.SyncInfo(on_wait=[w], on_update=[]),
                        ))
                    inst.sync_info = mybir.SyncInfo(
                        on_wait=keep,
                        on_update=list(si.on_update) if si.on_update else [],
                    )
                    changed = True
                new.append(inst)
            if changed:
                bb.instructions = new
    return ctr


def _build_program():
    nc = bass.Bass()
    xq = nc.declare_dram_parameter("xq", [D, S], BF16, isOutput=False)   # X_q^T
    xk = nc.declare_dram_parameter("xk", [D, S], BF16, isOutput=False)   # X_k^T
    xv = nc.declare_dram_parameter("xv", [D, S], BF16, isOutput=False)   # X_v^T
    wq = nc.declare_dram_parameter("wq", [D, D], BF16, isOutput=False)   # W_q/8
    wk = nc.declare_dram_parameter("wk", [D, D], BF16, isOutput=False)
    wv = nc.declare_dram_parameter("wv", [D, D], BF16, isOutput=False)
    msk = nc.declare_dram_parameter("msk", [128, KT], F32, isOutput=False)
    idn = nc.declare_dram_parameter("idn", [128, 128], BF16, isOutput=False)
    out = nc.declare_dram_parameter("out", [S, D], F32, isOutput=True)

    with tile.TileContext(nc) as tc:
        with (
            tc.tile_pool(name="persist", bufs=1) as pers,
            tc.tile_pool(name="xw", bufs=24) as xw,
            tc.tile_pool(name="pt", bufs=14) as ptp,
            tc.tile_pool(name="outp", bufs=3) as outp,
            tc.tile_pool(name="rr", bufs=8) as rrp,
            tc.tile_pool(name="pp", bufs=2, space="PSUM") as pp,
            tc.tile_pool(name="psc", bufs=2, space="PSUM") as psc,
            tc.tile_pool(name="pod", bufs=2, space="PSUM") as pod,
        ):
            # ---------- constants ----------
            mask_sb = pers.tile([128, KT], F32)
            nc.sync.dma_start(out=mask_sb, in_=msk[:, :])
            id_sb = pers.tile([128, 128], BF16)
            nc.sync.dma_start(out=id_sb, in_=idn[:, :])
            warm = pers.tile([128, 1], F32)
            nc.scalar.copy(warm, mask_sb[:, 0:1])            # warm ACT clock
            # HAM warm-up: junk matmuls on the identity tile while input DMAs
            # stream, so the first projection matmuls run at 2.4 GHz
            for _ in range(4):
                jw = pp.tile([32, 128], F32, tag="pp")
                for j in range(9):
                    nc.tensor.matmul(
                        jw[:, :], lhsT=id_sb[0:32, 0:32],
                        rhs=id_sb[0:32, 0:128],
                        start=(j == 0), stop=(j == 8))

            # ---------- persistent activations ----------
            QT = pers.tile([128, KT, S], BF16)     # Q^T tiles: rows 128r+p
            KTt = pers.tile([128, KT, S], BF16)    # K^T
            VA = pers.tile([128, KT, H * 65], BF16)  # V with ones columns

            def load_quarter(param, qtr):
                t = xw.tile([128, 2, S], BF16, tag="xw",
                            name=f"ld_{param.name}_{qtr}")
                nc.scalar.dma_start(
                    out=t, in_=param[:, :].rearrange(
                        "(a p) s -> p a s", p=128)[:, 2 * qtr:2 * qtr + 2, :])
                return t

            # Q/K activations+weights stream first (the scores->exp pipeline
            # is the pacer); V last (attnV tolerates lag via pt buffering).
            # The DMA engines round-robin packets across ALL enqueued
            # descriptors, so ungated pieces would all complete only at the
            # very end of the 12MB stream.  Gate: issue from the scalar
            # engine (idle until the first exp), 3 groups of 1MB in flight —
            # a tiny scalar copy READS group i-3 before group i is enqueued,
            # forcing sequential completion at full bandwidth.
            junk_g = pers.tile([128, 1], BF16)
            junk_s = pers.tile([1, 2], BF16)
            pieces = {}
            glast = []
            # Q/K groups: issued+gated on the scalar engine (idle until the
            # first exp, and all its gates clear before scores need K).
            for gi, (pmx, pmw) in enumerate([(xq, wq)] * 4 + [(xk, wk)] * 4):
                qtr = gi % 4
                if gi >= 3:
                    nc.scalar.copy(junk_g, glast[gi - 3][:, 0, 0:1])
                pieces[(pmx.name, qtr)] = load_quarter(pmx, qtr)
                pieces[(pmw.name, qtr)] = load_quarter(pmw, qtr)
                glast.append(pieces[(pmw.name, qtr)])
            # V groups: issued on gpsimd (idle until the first output DMA)
            # with gpsimd COMPUTE gates — a tensor_copy read waits for the
            # gated group's DMA DATA (a junk-DMA gate only waits for the
            # descriptor issue, which is useless).  First gate holds V until
            # the K stream has landed; then 2 groups in flight.
            def load_quarter_gp(param, qtr):
                t = xw.tile([128, 2, S], BF16, tag="xw",
                            name=f"ld_{param.name}_{qtr}")
                nc.gpsimd.dma_start(
                    out=t, in_=param[:, :].rearrange(
                        "(a p) s -> p a s", p=128)[:, 2 * qtr:2 * qtr + 2, :])
                return t

            for qtr in range(4):
                if qtr == 0:
                    nc.gpsimd.tensor_copy(junk_s, glast[7][0:1, 0, 0:2])
                elif qtr >= 2:
                    nc.gpsimd.tensor_copy(
                        junk_s, pieces[("wv", qtr - 2)][0:1, 0, 0:2])
                pieces[("xv", qtr)] = load_quarter_gp(xv, qtr)
                pieces[("wv", qtr)] = load_quarter_gp(wv, qtr)

            def mk_sl(param):
                ts4 = [pieces[(param.name, q)] for q in range(4)]
                def sl(k, cols, ts4=ts4):
                    return ts4[k // 2][:, k % 2, cols]
                return sl

            xv_s, wv_s = mk_sl(xv), mk_sl(wv)
            xq_s, wq_s = mk_sl(xq), mk_sl(wq)
            xk_s, wk_s = mk_sl(xk), mk_sl(wk)

            def va_slices(st, dc):
                dst = VA[:, st, :].rearrange("p (h w) -> p h w", w=65)
                return dst[:, dc * 8:(dc + 1) * 8, 0:64]

            def proj_qk(r, which, sc):
                w_s, x_s, dstT = ((wq_s, xq_s, QT) if which == 0
                                  else (wk_s, xk_s, KTt))
                pq = pp.tile([128, 512], F32, tag="pp")
                for k in range(KT):
                    nc.tensor.matmul(
                        pq[:, :],
                        lhsT=w_s(k, bass.ts(r, 128)),
                        rhs=x_s(k, bass.ts(sc, 512)),
                        start=(k == 0), stop=(k == KT - 1))
                nc.vector.tensor_copy(dstT[:, r, bass.ts(sc, 512)], pq)

            class ProjStepper:
                """Emit projection chains one matmul at a time so they pace
                evenly between attention chunks.

                Chain specs:
                  ('qk', which, sc, r)   -- 8 k-steps, full contraction
                  ('v', st, dc, half)    -- 4 k-steps (k = 4*half + j);
                     half 0 copies the partial sum into VA (bf16),
                     half 1 adds its partial sum on top (DVE tensor_add).
                """

                def __init__(self, chains, pool, tag, max_active=1):
                    self.pending = list(chains)
                    self.active = []   # [psum_tile, chain_spec, next_j]
                    self.rr = 0
                    self.pool, self.tag = pool, tag
                    self.MAX_ACTIVE = max_active

                def _start(self):
                    if self.pending:
                        spec = self.pending.pop(0)
                        pq = self.pool.tile([128, 512], F32, tag=self.tag,
                                            name=f"pq_{'_'.join(map(str, spec))}")
                        self.active.append([pq, spec, 0])

                def step(self, n=2):
                    for _ in range(n):
                        # keep TWO chains in flight and alternate their
                        # k-steps: consecutive matmuls accumulating into the
                        # SAME psum bank serialize fill/drain (~650ns/MM);
                        # alternating banks restores ~230ns/MM pipelining
                        while len(self.active) < self.MAX_ACTIVE and self.pending:
                            self._start()
                        if not self.active:
                            return
                        ent = self.active[self.rr % len(self.active)]
                        self.rr += 1
                        pq, spec, j = ent
                        if spec[0] == 'qk':
                            _, which, sc, r = spec
                            w_s, x_s = ((wq_s, xq_s) if which == 0
                                        else (wk_s, xk_s))
                            nsteps = KT
                            nc.tensor.matmul(
                                pq[:, :],
                                lhsT=w_s(j, bass.ts(r, 128)),
                                rhs=x_s(j, bass.ts(sc, 512)),
                                start=(j == 0), stop=(j == nsteps - 1))
                        else:
                            _, st, dc, half = spec
                            nsteps = 4
                            k = 4 * half + j
                            nc.tensor.matmul(
                                pq[:, :],
                                lhsT=xv_s(k, bass.ts(st, 128)),
                                rhs=wv_s(k, bass.ts(dc, 512)),
                                start=(j == 0), stop=(j == nsteps - 1))
                        ent[2] += 1
                        if ent[2] == nsteps:
                            if spec[0] == 'qk':
                                _, which, sc, r = spec
                                dstT = QT if which == 0 else KTt
                                nc.vector.tensor_copy(
                                    dstT[:, r, bass.ts(sc, 512)], pq)
                            else:
                                _, st, dc, half = spec
                                dst = va_slices(st, dc)
                                src = pq[:, :].rearrange(
                                    "p (h w) -> p h w", w=64)
                                if half == 0:
                                    nc.vector.tensor_copy(dst, src)
                                else:
                                    nc.vector.tensor_add(dst, dst, src)
                                    if dc == 1:
                                        ones = VA[:, st, :].rearrange(
                                            "p (h w) -> p h w", w=65)
                                        nc.vector.memset(
                                            ones[:, :, 64:65], 1.0)
                            self.active.remove(ent)

                def finish(self):
                    while self.active or self.pending:
                        self.step(1)

            # prelude: QT/KTt tile 0 ONLY — chain k-steps on the same PSUM
            # bank serialize (~650ns/MM); a 2-active stepper alternates two
            # banks so consecutive matmuls pipeline (~230ns).  Pair-1 chains
            # are paced inside pair 0.
            pre = ProjStepper([('qk', which, sc, 0)
                               for which in (0, 1) for sc in (0, 1)],
                              pp, "pp", max_active=2)
            pre.finish()

            OPs = {(0, 0): None, (0, 1): None, (1, 0): None, (1, 1): None}

            def scores_exp(r, qc, c):
                ps = psc.tile([128, 1024], F32, tag="psc")
                nc.tensor.matmul(
                    ps[:, 0:512],
                    lhsT=KTt[0:64, r, bass.ts(c, 128)],
                    rhs=QT[0:64, r, bass.ts(qc, 512)],
                    start=True, stop=True, tile_position=(0, 0))
                nc.tensor.matmul(
                    ps[:, 512:1024],
                    lhsT=KTt[64:128, r, bass.ts(c, 128)],
                    rhs=QT[64:128, r, bass.ts(qc, 512)],
                    start=True, stop=True, tile_position=(64, 0))
                pt = ptp.tile([128, 1024], BF16, tag="pt")
                nc.scalar.activation(pt, ps, AF.Exp,
                                     bias=mask_sb[:, c:c + 1], scale=1.0)
                return pt

            def attnv_chunk(r, po1, po2, pt, c):
                # attnV: pt-block stationary, [V_h | ones] moving. The 8
                # LDWEIGHTS pipeline into the PE background weight buffer,
                # so the whole burst issues in ~214ns.
                for ph, (po, hh) in enumerate(((po1, 2 * r), (po2, 2 * r + 1))):
                    for qt in range(4):
                        # start=True clears the WHOLE PSUM bank, so only the
                        # first slice-write of the group may set it; qt>0
                        # fresh-writes via per-element has_written instead.
                        nc.tensor.matmul(
                            po[:, qt, :],
                            lhsT=pt[:, ph * 512 + qt * 128:
                                    ph * 512 + (qt + 1) * 128],
                            rhs=VA[:, c, hh * 65:(hh + 1) * 65],
                            start=(c == 0 and qt == 0),
                            stop=(c == KT - 1))

            def outphase(r, qc, po1, po2):
                # denominator reciprocal + scale. Results for 4 consecutive
                # pairs are accumulated into one bf16 OP tile (512 output
                # columns -> 1KB DMA bursts; gpsimd DMA casts to fp32).
                rh = r // 4
                OP = OPs[(rh, qc)]
                if OP is None:
                    OP = OPs[(rh, qc)] = outp.tile(
                        [128, 4, 512], BF16, tag="outp", name=f"OP{rh}_{qc}")
                tail = (r == 7 and qc == 1)
                # The po psum banks are aliased by the NEXT group's attnV
                # (pod bufs=2 with 2 tiles/group = zero double buffering), so
                # drain them FAST: one bulk DVE copy each to SBUF (~0.25us),
                # then normalize out of SBUF on the idle gpsimd engine
                # (gpsimd cannot read PSUM directly).
                sb1 = rrp.tile([128, 4, 65], F32, tag="sbp", bufs=4,
                               name=f"sb1_{r}_{qc}")
                sb2 = rrp.tile([128, 4, 65], F32, tag="sbp", bufs=4,
                               name=f"sb2_{r}_{qc}")
                nc.vector.tensor_copy(sb1, po1)
                nc.vector.tensor_copy(sb2, po2)
                rr1 = rrp.tile([128, 4, 1], F32, tag="rr", bufs=4,
                               name=f"rr1_{r}_{qc}")
                rr2 = rrp.tile([128, 4, 1], F32, tag="rr", bufs=4,
                               name=f"rr2_{r}_{qc}")
                nc.vector.reciprocal(rr1, sb1[:, :, 64:65])
                nc.vector.reciprocal(rr2, sb2[:, :, 64:65])
                for qt in range(4):
                    for ph, (sb, rr) in enumerate(((sb1, rr1), (sb2, rr2))):
                        nc.gpsimd.tensor_scalar_mul(
                            OP[:, qt, (r % 4) * 128 + ph * DH:
                               (r % 4) * 128 + (ph + 1) * DH],
                            sb[:, qt, 0:64], rr[:, qt, 0:1])
                    if tail:
                        # last tile: DMA row-block by row-block as the
                        # divisions finish, to shorten the exposed tail
                        nc.gpsimd.dma_start(
                            out=out[qc * 512 + qt * 128:
                                    qc * 512 + (qt + 1) * 128,
                                    bass.ts(rh, 512)],
                            in_=OP[:, qt, :])
                if r % 4 == 3 and not tail:
                    nc.gpsimd.dma_start(
                        out=out[bass.ts(qc, 512), bass.ts(rh, 512)].rearrange(
                            "(a p) w -> p a w", p=128),
                        in_=OP[:, :, :])
                    OPs[(rh, qc)] = None

            # ---- main loop: uniform software pipeline over 16 qc-groups.
            # The exp stream (the ACT pacer) runs continuously; attnV for
            # group g-1 interleaves into group g's steps using pts held from
            # the previous group (peak 9 live pt tiles).  V chains pace into
            # groups 0 (A halves, k 0-3) and 1 (B halves, k 4-7, each
            # completing VA st=c right before attnv(group0, c) needs it).
            # Pair r+1's QK chains pace at 2/step through pair r's steps.
            stepA = ProjStepper([('v', st, dc, 0)
                                 for st in range(8) for dc in range(2)],
                                pod, "pod")
            stepB = ProjStepper([('v', st, dc, 1)
                                 for st in range(8) for dc in range(2)],
                                pp, "pp")
            qk_step = {}
            for rr_ in range(1, 8):
                qk_step[rr_] = ProjStepper([('qk', which, sc, rr_)
                                            for which in (0, 1)
                                            for sc in (0, 1)],
                                           pp, "pp")
            held = None          # (r, qc, pts) of the previous group
            for g in range(16):
                r, qc = g // 2, g % 2
                if held is not None:
                    hp1 = pod.tile([128, 4, 65], F32, tag="pod",
                                   name=f"po1_g{g - 1}")
                    hp2 = pod.tile([128, 4, 65], F32, tag="pod",
                                   name=f"po2_g{g - 1}")
                pts = []
                for c in range(KT):
                    pts.append(scores_exp(r, qc, c))
                    if g == 0:
                        stepA.step(8)        # pod banks (attnV idle)
                        qk_step[1].step(4)   # pp banks
                    elif g == 1:
                        stepB.step(8)        # pp banks (qk1 exhausted)
                    elif r < 7:
                        # finish the next pair's chains by mid-group so the
                        # QT/KTt copies land well before its first scores
                        qk_step[r + 1].step(
                            2 if qc == 0 else
                            (3 if c < 4 else (2 if c < 6 else 0)))
                    if held is not None:
                        attnv_chunk(held[0], hp1, hp2, held[2][c], c)
                if held is not None:
                    outphase(held[0], held[1], hp1, hp2)
                held = (r, qc, pts)
            # drain: last group's attnV + out-phase
            hp1 = pod.tile([128, 4, 65], F32, tag="pod", name="po1_g15")
            hp2 = pod.tile([128, 4, 65], F32, tag="pod", name="po2_g15")
            for c in range(KT):
                attnv_chunk(held[0], hp1, hp2, held[2][c], c)
            outphase(held[0], held[1], hp1, hp2)

    _split_excess_waits(nc)
    return nc


def _prep_inputs(queries, keys, values, valid_lens, w_q, w_k, w_v):
    bf = ml_dtypes.bfloat16
    wq_b = np.ascontiguousarray((w_q.astype(np.float32) / np.sqrt(DH)).astype(bf))
    wk_b = np.ascontiguousarray(w_k.astype(np.float32).astype(bf))
    wv_b = np.ascontiguousarray(w_v.astype(np.float32).astype(bf))
    idn = np.eye(128, dtype=bf)
    in_maps = []
    for b in range(B):
        mask = np.where(np.arange(S) < int(valid_lens[b]), 0.0, NEG)
        mask = np.ascontiguousarray(
            mask.reshape(KT, 128).T.astype(np.float32))          # [128, KT]
        in_maps.append(dict(
            xq=np.ascontiguousarray(queries[b].astype(np.float32).T.astype(bf)),
            xk=np.ascontiguousarray(keys[b].astype(np.float32).T.astype(bf)),
            xv=np.ascontiguousarray(values[b].astype(np.float32).T.astype(bf)),
            wq=wq_b, wk=wk_b, wv=wv_b, msk=mask, idn=idn,
        ))
    return in_maps


def kernel(queries, keys, values, valid_lens, w_q, w_k, w_v, _want_results=False):
    queries = np.asarray(queries)
    keys = np.asarray(keys)
    values = np.asarray(values)
    valid_lens = np.asarray(valid_lens)
    w_q, w_k, w_v = np.asarray(w_q), np.asarray(w_k), np.asarray(w_v)
    if "nc" not in _cache:
        _cache["nc"] = _build_program()
    nc = _cache["nc"]
    in_maps = _prep_inputs(queries, keys, values, valid_lens, w_q, w_k, w_v)
    res = run_bass_kernel_spmd(nc, in_maps, list(range(N_CORES)))
    out = np.stack([res.results[b]["out"] for b in range(B)]).astype(np.float32)
    # valid_len == 0: reference softmaxes an all -1e9 row -> uniform attention.
    for b in range(B):
        if int(valid_lens[b]) == 0:
            vfull = values[b].astype(np.float32) @ w_v.astype(np.float32)
            out[b] = np.broadcast_to(vfull.mean(axis=0), (S, D))
    if _want_results:
        return out, res
    return out

